# revision 3
# baseline (speedup 1.0000x reference)
"""Trainium2 Bass kernel for the boundary loss:

    loss = mean_b mean_hw( |sigmoid(logits) - targets| * EDT(targets) )

where EDT is the exact Euclidean distance transform of the background.

Pipeline (per sample, H=W=384, bf16 throughout):
  1. Row pass: exact 1D row distances g via two chamfer scans per row
     (tensor_tensor_scan: state = min(state+1, B0[t]); the backward pass
     runs on the forward result through reversed access patterns). Exact
     for any distance, so no window radius is needed along W.
  2. PE transposes g; the PSUM evacuation applies Square on the ACT
     engine, producing g^2 in transposed layout (h along the free dim)
     with R pad columns preset to BIG.
  3. Column pass: windowed min-plus over |dh| <= R
     (d2 = min_dh dh^2 + g2[h+dh]) using per-offset bias tiles
     (tensor_scalar, 4x DVE mode; the d=2 bias builds on the otherwise
     idle GpSimd engine) and tensor_tensor mins (2x DVE mode). R comes
     from a sound host-side validation: if the exact max d2 < (R+1)^2,
     every pixel's optimum lies inside the window, so the windowed result
     is the exact EDT. Random 0/1 targets give R = 2.
  4. PE transposes back; the evacuation applies Sqrt -> dist.
  5. |sigmoid(x)-t|*dist == sigmoid(x)*dist (dist is 0 where t=1), so the
     loss is sum(sigmoid(x)*dist): per-row-block scalar_tensor_tensor
     products with accum_out columns, summed on the host.

All values that can win a min are small integers (<= 2R^2 for R <= 11),
exact in bf16; larger R falls back to an f32 build.

Sharding: data-parallel over batch, 2 samples per NeuronCore on 8 cores;
each core emits per-(partition, r-block) sums, the host adds them up.
"""
import os
import sys

sys.path.insert(0, "/opt/trn_rl_repo")

import numpy as np

import concourse.bass as bass
from concourse import masks, mybir
from concourse.bass_utils import run_bass_kernel_spmd
from concourse.tile import TileContext, ScopedClock

F32 = mybir.dt.float32
BF16 = mybir.dt.bfloat16
AF = mybir.ActivationFunctionType
OP = mybir.AluOpType

N_CORES = 8
B, H, W = 16, 384, 384
SPC = B // N_CORES  # samples per core
P = 128
HT = H // P  # 128-row blocks per sample
NF = HT * W  # free elements per fused tile
REF_BIG = float(H + W)  # reference clips distances to this for fg-free samples

LAST_RESULTS = None

# ---------------------------------------------------------------------------
# Walrus in this container rejects >1 sync-wait per instruction ("Too many
# sync wait commands").  Keep the last wait on the instruction and move the
# rest onto same-engine NOPs inserted right before it.
_UID = [0]


def _split_excess_waits(nc, max_waits=1):
    for f in nc.m.functions:
        for bb in f.blocks:
            out = []
            changed = False
            for inst in bb.instructions:
                si = getattr(inst, "sync_info", None)
                waits = list(si.on_wait) if si is not None and si.on_wait else []
                if len(waits) > max_waits:
                    for w in waits[:-max_waits]:
                        _UID[0] += 1
                        nop = mybir.InstNoOp(name=f"I-waitsplit-{_UID[0]}")
                        nop.engine = inst.engine
                        nop.sync_info = mybir.SyncInfo(on_wait=[w], on_update=[])
                        nc.register_instruction(nop)
                        out.append(nop)
                    inst.sync_info = mybir.SyncInfo(
                        on_wait=waits[-max_waits:],
                        on_update=list(si.on_update) if si.on_update else [],
                    )
                    changed = True
                out.append(inst)
            if changed:
                bb.instructions = out


def _split_drain_and_barrier(self, tick_clock, wait_clock):
    nc = self.nc
    drain_inst = nc.sync.drain()
    wait_clock.add_sem_waits(
        drain_inst.ins, ScopedClock({None: tick_clock.global_clock})
    )
    nc.all_engine_barrier()
    assert self.sems is not None
    popped = nc._tile_sem_poison_stack.pop()
    assert popped is self._sem_poison
    nc.clear_and_free_semaphores(list(self.sems.allocated().values()))
    nc.all_engine_barrier()
    _split_excess_waits(nc)


TileContext._drain_and_barrier = _split_drain_and_barrier
# ---------------------------------------------------------------------------


def _build(R):
    """Per-core SPMD kernel with column-window radius R."""
    EDT = BF16 if R <= 11 else F32
    BIG = 16384.0 if R <= 11 else 1.0e7
    WP = W + 2 * R  # padded transposed row length
    nc = bass.Bass("TRN2", target_bir_lowering=False, debug=False,
                   num_devices=N_CORES)
    lg = nc.dram_tensor("logits", [SPC, 1, H, W], BF16, kind="ExternalInput").ap()
    tg = nc.dram_tensor("targets", [SPC, 1, H, W], BF16, kind="ExternalInput").ap()
    o_sum = nc.dram_tensor("o_sum", [P, SPC * HT], F32,
                           kind="ExternalOutput").ap()

    def dram_tile(t, s):
        return t[s, 0].rearrange("(r p) w -> p r w", p=P)

    with TileContext(nc) as tc:
        with (
            tc.tile_pool(name="const", bufs=1) as cpool,
            tc.tile_pool(name="work", bufs=1) as wk,
            tc.tile_pool(name="psA", bufs=1, space="PSUM") as psA,
            tc.tile_pool(name="psB", bufs=1, space="PSUM") as psB,
        ):
            ident = cpool.tile([P, P], EDT, tag="ident", name="ident")
            masks.make_identity(nc, ident[:])
            ones = cpool.tile([P, W], EDT, tag="ones", name="ones")
            nc.gpsimd.memset(ones[:], 1.0)
            rowsum = cpool.tile([P, SPC * HT], F32, tag="rowsum", name="rowsum")

            # per-sample tiles (persistent; distinct tags)
            T = []
            for s in range(SPC):
                d = {}
                d["tg"] = wk.tile([P, NF], BF16, tag=f"tg{s}", name=f"tg{s}")
                d["x"] = wk.tile([P, NF], BF16, tag=f"x{s}", name=f"x{s}")
                d["b0"] = wk.tile([P, NF], EDT, tag=f"b0{s}", name=f"b0{s}")
                d["df"] = wk.tile([P, NF], EDT, tag=f"df{s}", name=f"df{s}")
                d["g"] = wk.tile([P, NF], EDT, tag=f"g{s}", name=f"g{s}")
                d["g2t"] = wk.tile([P, HT, WP], EDT, tag=f"g2t{s}",
                                   name=f"g2t{s}")
                d["bias"] = [
                    wk.tile([P, HT, WP], EDT, tag=f"bs{s}_{dd}",
                            name=f"bs{s}_{dd}")
                    for dd in range(1, R + 1)
                ]
                d["m"] = wk.tile([P, NF], EDT, tag=f"m{s}", name=f"m{s}")
                d["d2t"] = wk.tile([P, NF], EDT, tag=f"d2t{s}", name=f"d2t{s}")
                d["dist"] = wk.tile([P, NF], EDT, tag=f"dist{s}",
                                    name=f"dist{s}")
                d["sig"] = wk.tile([P, NF], BF16, tag=f"sig{s}", name=f"sig{s}")
                d["junk"] = wk.tile([P, NF], EDT, tag=f"junk{s}",
                                    name=f"junk{s}")
                d["psA"] = psA.tile([P, NF], EDT, tag=f"psA{s}", name=f"psA{s}")
                d["psB"] = psB.tile([P, NF], EDT, tag=f"psB{s}", name=f"psB{s}")
                # pads of g2t = BIG; bias-tile pads inherit BIG+d^2 via the
                # full-width bias builds
                nc.gpsimd.memset(d["g2t"][:][:, :, 0:R], BIG)
                nc.gpsimd.memset(d["g2t"][:][:, :, W + R:WP], BIG)
                T.append(d)

            # ---- input DMA: sample 0 targets row 0 first (pipeline head)
            nc.sync.dma_start(_r3(T[0]["tg"])[:, 0:1], dram_tile(tg, 0)[:, 0:1])
            nc.sync.dma_start(_r3(T[0]["tg"])[:, 1:HT], dram_tile(tg, 0)[:, 1:HT])
            nc.scalar.dma_start(_r3(T[1]["tg"]), dram_tile(tg, 1))
            nc.sync.dma_start(_r3(T[0]["x"]), dram_tile(lg, 0))
            nc.scalar.dma_start(_r3(T[1]["x"]), dram_tile(lg, 1))

            def b0_build(s, lo, hi):
                nc.vector.tensor_scalar(
                    _r3(T[s]["b0"])[:, lo:hi], _r3(T[s]["tg"])[:, lo:hi],
                    -BIG, BIG, OP.mult, OP.add)

            def fwd(s, r):
                sl = slice(r * W, (r + 1) * W)
                nc.vector.tensor_tensor_scan(
                    T[s]["df"][:][:, sl], ones[:], T[s]["b0"][:][:, sl],
                    BIG, OP.add, OP.min)

            def bwd(s, r):
                sl = slice(r * W, (r + 1) * W)
                nc.vector.tensor_tensor_scan(
                    T[s]["g"][:][:, sl][:, ::-1], ones[:],
                    T[s]["df"][:][:, sl][:, ::-1], BIG, OP.add, OP.min)

            def fwd_transpose_row(s, r):
                gv = _r3(T[s]["g"])
                psv = _r3h(T[s]["psA"])
                for c in range(HT):
                    nc.tensor.transpose(
                        psv[:, c, r * P:(r + 1) * P],
                        gv[:, r, c * P:(c + 1) * P], ident[:])

            def square_evac(s):
                nc.scalar.activation(
                    T[s]["g2t"][:][:, :, R:R + W], _r3h(T[s]["psA"]),
                    AF.Square)

            def bias_build(s, dd, eng):
                eng.tensor_scalar(
                    T[s]["bias"][dd - 1][:], T[s]["g2t"][:], float(dd * dd),
                    None, OP.add)

            def stage_b(s):
                """d2t = min_{|dh|<=R} dh^2 + g2t[h+dh]; biases built lazily
                so the d>=2 builds (GpSimd) overlap the d=1 mins (DVE)."""
                g2tc = T[s]["g2t"][:][:, :, R:R + W]
                mv = _r3h(T[s]["m"])
                d2v = _r3h(T[s]["d2t"])
                for dd in range(1, R + 1):
                    Bv = T[s]["bias"][dd - 1][:]
                    lo, hi = R - dd, R + dd
                    nc.vector.tensor_tensor(
                        mv[:] if dd == 1 else d2v[:],
                        Bv[:, :, lo:lo + W], Bv[:, :, hi:hi + W], OP.min)
                    if dd == 1:
                        nc.vector.tensor_tensor(mv[:], mv[:], g2tc, OP.min)
                    else:
                        nc.vector.tensor_tensor(d2v[:], mv[:], d2v[:], OP.min)
                        if dd < R:
                            nc.vector.tensor_copy(mv[:], d2v[:])
                if R == 1:
                    nc.vector.tensor_copy(d2v[:], mv[:])

            def back_transpose(s):
                d2v = _r3h(T[s]["d2t"])
                psv = _r3(T[s]["psB"])
                for r in range(HT):
                    for c in range(HT):
                        nc.tensor.transpose(
                            psv[:, r, c * P:(c + 1) * P],
                            d2v[:, c, r * P:(r + 1) * P], ident[:])

            def sqrt_evac(s, r):
                nc.scalar.activation(
                    _r3(T[s]["dist"])[:, r:r + 1], _r3(T[s]["psB"])[:, r:r + 1],
                    AF.Sqrt)

            def sigmoid(s):
                nc.scalar.activation(T[s]["sig"][:], T[s]["x"][:], AF.Sigmoid)

            def product(s, r):
                nc.vector.scalar_tensor_tensor(
                    _r3(T[s]["junk"])[:, r:r + 1], _r3(T[s]["sig"])[:, r:r + 1],
                    1.0, _r3(T[s]["dist"])[:, r:r + 1], OP.mult, OP.mult,
                    accum_out=rowsum[:, s * HT + r:s * HT + r + 1])

            # ---------------- schedule (issue order == engine order) -------
            # s0 head: per-row b0 + scans; transposes trail each bwd
            b0_build(0, 0, 1)
            fwd(0, 0)
            b0_build(0, 1, HT)
            bwd(0, 0)
            fwd_transpose_row(0, 0)
            fwd(0, 1)
            bwd(0, 1)
            fwd_transpose_row(0, 1)
            fwd(0, 2)
            bwd(0, 2)
            fwd_transpose_row(0, 2)
            square_evac(0)
            sigmoid(0)

            # s1 scans fill DVE while s0's evac runs on ACT
            b0_build(1, 0, HT)
            fwd(1, 0)
            bwd(1, 0)
            fwd(1, 1)

            # s0 stage B (d=1 bias on DVE; d>=2 biases on GpSimd)
            bias_build(0, 1, nc.vector)
            for dd in range(2, R + 1):
                bias_build(0, dd, nc.gpsimd)
            stage_b(0)
            back_transpose(0)
            for r in range(HT):
                sqrt_evac(0, r)

            # s1 scans finish
            bwd(1, 1)
            fwd(1, 2)
            bwd(1, 2)
            for r in range(HT):
                fwd_transpose_row(1, r)
            square_evac(1)
            sigmoid(1)

            # s0 products drain while s1 stage B builds
            for r in range(HT):
                product(0, r)
            nc.sync.dma_start(o_sum[:, 0:HT], rowsum[:, 0:HT])

            bias_build(1, 1, nc.vector)
            for dd in range(2, R + 1):
                bias_build(1, dd, nc.gpsimd)
            stage_b(1)
            back_transpose(1)
            for r in range(HT):
                sqrt_evac(1, r)
                product(1, r)
            nc.sync.dma_start(o_sum[:, HT:2 * HT], rowsum[:, HT:2 * HT])

    return nc


def _r3(ap_tile):
    return ap_tile[:].rearrange("p (r w) -> p r w", w=W)


def _r3h(ap_tile):
    return ap_tile[:].rearrange("p (c h) -> p c h", h=W)


_KERNEL_CACHE = {}


def _get_kernel(R):
    if R not in _KERNEL_CACHE:
        _KERNEL_CACHE[R] = _build(R)
    return _KERNEL_CACHE[R]


def _exact_row_dist(fg):
    """Exact 1D row distances (distance to nearest fg in the same row),
    float64, BIG-ish large where a row has no fg. fg: [B, H, W] bool."""
    Bn, Hn, Wn = fg.shape
    BIGV = 1.0e9
    col = np.arange(Wn, dtype=np.float64)
    left = np.where(fg, col, -BIGV)
    np.maximum.accumulate(left, axis=2, out=left)
    d_left = col[None, None, :] - left
    right = np.where(fg, -col, -BIGV)[:, :, ::-1]
    np.maximum.accumulate(right, axis=2, out=right)
    d_right = (-right[:, :, ::-1]) - col[None, None, :]
    return np.minimum(d_left, d_right)


def _pick_R(fg):
    """Smallest column-window radius R whose windowed pass equals the exact
    EDT, verified by the sound criterion max(d2_R) < (R+1)^2 (then every
    pixel's optimal |dh| < R+1, so the window covers the true optimum)."""
    g = _exact_row_dist(fg)
    g2 = g * g
    R = 2
    while True:
        d2 = g2.copy()
        for d in range(1, R + 1):
            dd = float(d * d)
            d2[:, :H - d, :] = np.minimum(d2[:, :H - d, :], g2[:, d:, :] + dd)
            d2[:, d:, :] = np.minimum(d2[:, d:, :], g2[:, :H - d, :] + dd)
        if d2.max() < (R + 1) ** 2 or R >= H - 1:
            return R
        R = min(max(R * 2, R + 1), H - 1)


def kernel(logits, targets):
    logits = np.ascontiguousarray(np.asarray(logits, dtype=np.float32))
    targets = np.ascontiguousarray(np.asarray(targets, dtype=np.int32))

    fg = targets[:, 0] > 0
    host_extra = 0.0
    empty = ~fg.any(axis=(1, 2))
    if empty.any():
        # no foreground anywhere: the reference's clipped row-scan gives
        # dist(i,j) = H+W - j. Contribute |sigmoid - 0| * dist on the host
        # and neutralize the sample on device (all-fg -> dist 0).
        dist_empty = REF_BIG - np.arange(W, dtype=np.float64)[None, :]
        for s in np.nonzero(empty)[0]:
            p = 1.0 / (1.0 + np.exp(-logits[s, 0].astype(np.float64)))
            host_extra += float((p * dist_empty).sum())
        targets = targets.copy()
        targets[empty] = 1
        fg = targets[:, 0] > 0

    R = _pick_R(fg)
    import ml_dtypes

    targets_bf16 = np.ascontiguousarray(targets.astype(ml_dtypes.bfloat16))
    logits_bf16 = np.ascontiguousarray(logits.astype(ml_dtypes.bfloat16))
    trace = bool(os.environ.get("BASS_TRACE"))
    nc = _get_kernel(R)
    in_maps = [
        {
            "logits": logits_bf16[i * SPC:(i + 1) * SPC],
            "targets": targets_bf16[i * SPC:(i + 1) * SPC],
        }
        for i in range(N_CORES)
    ]
    res = run_bass_kernel_spmd(nc, in_maps, core_ids=list(range(N_CORES)),
                               trace=trace)
    global LAST_RESULTS
    LAST_RESULTS = res

    total = sum(
        float(np.asarray(r["o_sum"], dtype=np.float64).sum())
        for r in res.results
    ) + host_extra
    return np.float32(total / (B * H * W))


# revision 4
# speedup vs baseline: 1.0881x; 1.0881x over previous
"""Trainium2 Bass kernel for the boundary loss:

    loss = mean_b mean_hw( |sigmoid(logits) - targets| * EDT(targets) )

where EDT is the exact Euclidean distance transform of the background.

Pipeline (per sample, H=W=384, bf16 throughout):
  1. Row pass: exact 1D row distances g via two chamfer scans per row
     (tensor_tensor_scan: state = min(state+1, B0[t]); the backward pass
     runs on the forward result through reversed access patterns). Exact
     for any distance, so no window radius is needed along W.
  2. PE transposes g; the PSUM evacuation applies Square on the ACT
     engine, producing g^2 in transposed layout (h along the free dim)
     with R pad columns preset to BIG.
  3. Column pass: windowed min-plus over |dh| <= R
     (d2 = min_dh dh^2 + g2[h+dh]) using per-offset bias tiles
     (tensor_scalar, 4x DVE mode; d>=2 builds on the otherwise idle
     GpSimd engine) and tensor_tensor mins (2x DVE mode), split per
     128-column block so the chain pipelines. R comes from a sound
     host-side validation: if the exact max d2 < (R+1)^2, every pixel's
     optimum lies inside the window, so the windowed result is the exact
     EDT. Random 0/1 targets give R = 2.
  4. The product stays in transposed space (sum is layout-invariant):
     logits are PE-transposed too, sigmoid rides the PSUM evacuation,
     dist = ACT sqrt of d2, and per-column-block scalar_tensor_tensor
     products emit accum columns summed on the host. No back-transpose.

All values that can win a min are small integers (<= 2R^2 for R <= 11),
exact in bf16; larger R falls back to an f32 build.

Sharding: data-parallel over batch, 2 samples per NeuronCore on 8 cores;
each core emits per-(partition, c-block) sums, the host adds them up.
"""
import os
import sys

sys.path.insert(0, "/opt/trn_rl_repo")

import numpy as np

import concourse.bass as bass
from concourse import masks, mybir
from concourse.bass_utils import run_bass_kernel_spmd
from concourse.tile import TileContext, ScopedClock

F32 = mybir.dt.float32
BF16 = mybir.dt.bfloat16
AF = mybir.ActivationFunctionType
OP = mybir.AluOpType

N_CORES = 8
B, H, W = 16, 384, 384
SPC = B // N_CORES  # samples per core
P = 128
HT = H // P  # 128-row blocks per sample
NF = HT * W  # free elements per fused tile
REF_BIG = float(H + W)  # reference clips distances to this for fg-free samples

LAST_RESULTS = None

# ---------------------------------------------------------------------------
# Walrus in this container rejects >1 sync-wait per instruction ("Too many
# sync wait commands").  Keep the last wait on the instruction and move the
# rest onto same-engine NOPs inserted right before it.
_UID = [0]


def _split_excess_waits(nc, max_waits=1):
    for f in nc.m.functions:
        for bb in f.blocks:
            out = []
            changed = False
            for inst in bb.instructions:
                si = getattr(inst, "sync_info", None)
                waits = list(si.on_wait) if si is not None and si.on_wait else []
                if len(waits) > max_waits:
                    for w in waits[:-max_waits]:
                        _UID[0] += 1
                        nop = mybir.InstNoOp(name=f"I-waitsplit-{_UID[0]}")
                        nop.engine = inst.engine
                        nop.sync_info = mybir.SyncInfo(on_wait=[w], on_update=[])
                        nc.register_instruction(nop)
                        out.append(nop)
                    inst.sync_info = mybir.SyncInfo(
                        on_wait=waits[-max_waits:],
                        on_update=list(si.on_update) if si.on_update else [],
                    )
                    changed = True
                out.append(inst)
            if changed:
                bb.instructions = out


def _split_drain_and_barrier(self, tick_clock, wait_clock):
    nc = self.nc
    drain_inst = nc.sync.drain()
    wait_clock.add_sem_waits(
        drain_inst.ins, ScopedClock({None: tick_clock.global_clock})
    )
    nc.all_engine_barrier()
    assert self.sems is not None
    popped = nc._tile_sem_poison_stack.pop()
    assert popped is self._sem_poison
    nc.clear_and_free_semaphores(list(self.sems.allocated().values()))
    nc.all_engine_barrier()
    _split_excess_waits(nc)


TileContext._drain_and_barrier = _split_drain_and_barrier
# ---------------------------------------------------------------------------


def _build(R):
    """Per-core SPMD kernel with column-window radius R."""
    EDT = BF16 if R <= 11 else F32
    BIG = 16384.0 if R <= 11 else 1.0e7
    WP = W + 2 * R  # padded transposed row length
    nc = bass.Bass("TRN2", target_bir_lowering=False, debug=False,
                   num_devices=N_CORES)
    lg = nc.dram_tensor("logits", [SPC, 1, H, W], BF16, kind="ExternalInput").ap()
    tg = nc.dram_tensor("targets", [SPC, 1, H, W], BF16, kind="ExternalInput").ap()
    o_sum = nc.dram_tensor("o_sum", [P, SPC * HT], F32,
                           kind="ExternalOutput").ap()

    def dram_tile(t, s):
        return t[s, 0].rearrange("(r p) w -> p r w", p=P)

    def r3(tile):  # [P, (r w)] -> [P, r, w]
        return tile[:].rearrange("p (r w) -> p r w", w=W)

    with TileContext(nc) as tc:
        with (
            tc.tile_pool(name="const", bufs=1) as cpool,
            tc.tile_pool(name="work", bufs=1) as wk,
            tc.tile_pool(name="psA", bufs=1, space="PSUM") as psA,
            tc.tile_pool(name="psX", bufs=1, space="PSUM") as psX,
        ):
            ident = cpool.tile([P, P], EDT, tag="ident", name="ident")
            masks.make_identity(nc, ident[:])
            ones = cpool.tile([P, W], EDT, tag="ones", name="ones")
            nc.gpsimd.memset(ones[:], 1.0)
            rowsum = cpool.tile([P, SPC * HT], F32, tag="rowsum", name="rowsum")

            T = []
            for s in range(SPC):
                d = {}
                d["tg"] = wk.tile([P, NF], BF16, tag=f"tg{s}", name=f"tg{s}")
                d["x"] = wk.tile([P, NF], BF16, tag=f"x{s}", name=f"x{s}")
                d["b0"] = wk.tile([P, NF], EDT, tag=f"b0{s}", name=f"b0{s}")
                d["df"] = wk.tile([P, NF], EDT, tag=f"df{s}", name=f"df{s}")
                d["g"] = wk.tile([P, NF], EDT, tag=f"g{s}", name=f"g{s}")
                d["g2t"] = wk.tile([P, HT, WP], EDT, tag=f"g2t{s}",
                                   name=f"g2t{s}")
                d["bias"] = [
                    wk.tile([P, HT, WP], EDT, tag=f"bs{s}_{dd}",
                            name=f"bs{s}_{dd}")
                    for dd in range(1, R + 1)
                ]
                d["m"] = wk.tile([P, NF], EDT, tag=f"m{s}", name=f"m{s}")
                d["d2t"] = wk.tile([P, NF], EDT, tag=f"d2t{s}", name=f"d2t{s}")
                d["dist"] = wk.tile([P, NF], EDT, tag=f"dist{s}",
                                    name=f"dist{s}")
                d["sigt"] = wk.tile([P, NF], BF16, tag=f"sig{s}",
                                    name=f"sig{s}")
                d["junk"] = wk.tile([P, NF], EDT, tag=f"junk{s}",
                                    name=f"junk{s}")
                d["psA"] = psA.tile([P, NF], EDT, tag=f"psA{s}", name=f"psA{s}")
                d["psX"] = psX.tile([P, NF], BF16, tag=f"psX{s}",
                                    name=f"psX{s}")
                nc.gpsimd.memset(d["g2t"][:][:, :, 0:R], BIG)
                nc.gpsimd.memset(d["g2t"][:][:, :, W + R:WP], BIG)
                T.append(d)

            # ---- input DMA: per-row targets for sample 0 (pipeline head)
            for r in range(HT):
                nc.sync.dma_start(r3(T[0]["tg"])[:, r:r + 1],
                                  dram_tile(tg, 0)[:, r:r + 1])
            nc.scalar.dma_start(r3(T[1]["tg"]), dram_tile(tg, 1))
            nc.sync.dma_start(r3(T[0]["x"]), dram_tile(lg, 0))
            nc.scalar.dma_start(r3(T[1]["x"]), dram_tile(lg, 1))

            def b0_row(s, r):
                nc.vector.tensor_scalar(
                    r3(T[s]["b0"])[:, r:r + 1], r3(T[s]["tg"])[:, r:r + 1],
                    -BIG, BIG, OP.mult, OP.add)

            def b0_act(s):
                nc.scalar.activation(T[s]["b0"][:], T[s]["tg"][:], AF.Copy,
                                     bias=BIG, scale=-BIG)

            def fwd(s, r):
                sl = slice(r * W, (r + 1) * W)
                nc.vector.tensor_tensor_scan(
                    T[s]["df"][:][:, sl], ones[:], T[s]["b0"][:][:, sl],
                    BIG, OP.add, OP.min)

            def bwd(s, r):
                sl = slice(r * W, (r + 1) * W)
                nc.vector.tensor_tensor_scan(
                    T[s]["g"][:][:, sl][:, ::-1], ones[:],
                    T[s]["df"][:][:, sl][:, ::-1], BIG, OP.add, OP.min)

            def g_transpose_row(s, r):
                gv = r3(T[s]["g"])
                psv = r3(T[s]["psA"])  # [p, c, h]
                for c in range(HT):
                    nc.tensor.transpose(
                        psv[:, c, r * P:(r + 1) * P],
                        gv[:, r, c * P:(c + 1) * P], ident[:])

            def x_transpose(s):
                xv = r3(T[s]["x"])
                psv = r3(T[s]["psX"])
                for c in range(HT):
                    for r in range(HT):
                        nc.tensor.transpose(
                            psv[:, c, r * P:(r + 1) * P],
                            xv[:, r, c * P:(c + 1) * P], ident[:])

            def square_evac(s):
                nc.scalar.activation(
                    T[s]["g2t"][:][:, :, R:R + W], r3(T[s]["psA"]), AF.Square)

            def sig_evac(s):
                nc.scalar.activation(r3(T[s]["sigt"]), r3(T[s]["psX"]),
                                     AF.Sigmoid)

            def bias_build(s, dd, eng):
                if eng is nc.scalar:
                    nc.scalar.activation(T[s]["bias"][dd - 1][:],
                                         T[s]["g2t"][:], AF.Copy,
                                         bias=float(dd * dd))
                else:
                    eng.tensor_scalar(
                        T[s]["bias"][dd - 1][:], T[s]["g2t"][:],
                        float(dd * dd), None, OP.add)

            def stage_b_col(s, c):
                """d2t[:, c, :] = min_{|dh|<=R} dh^2 + g2t[c, h+dh]."""
                g2tc = T[s]["g2t"][:][:, c:c + 1, R:R + W]
                mv = r3(T[s]["m"])[:, c:c + 1]
                d2v = r3(T[s]["d2t"])[:, c:c + 1]
                for dd in range(1, R + 1):
                    Bv = T[s]["bias"][dd - 1][:][:, c:c + 1]
                    lo, hi = R - dd, R + dd
                    nc.vector.tensor_tensor(
                        mv if dd == 1 else d2v,
                        Bv[:, :, lo:lo + W], Bv[:, :, hi:hi + W], OP.min)
                    if dd == 1:
                        nc.vector.tensor_tensor(mv, mv, g2tc, OP.min)
                    else:
                        nc.vector.tensor_tensor(d2v, mv, d2v, OP.min)
                        if dd < R:
                            nc.vector.tensor_copy(mv, d2v)
                if R == 1:
                    nc.vector.tensor_copy(d2v, mv)

            def sqrt_col(s, c):
                nc.scalar.activation(
                    r3(T[s]["dist"])[:, c:c + 1], r3(T[s]["d2t"])[:, c:c + 1],
                    AF.Sqrt)

            def product_col(s, c):
                nc.vector.scalar_tensor_tensor(
                    r3(T[s]["junk"])[:, c:c + 1], r3(T[s]["sigt"])[:, c:c + 1],
                    1.0, r3(T[s]["dist"])[:, c:c + 1], OP.mult, OP.mult,
                    accum_out=rowsum[:, s * HT + c:s * HT + c + 1])

            # ---------------- schedule (issue order biases the scheduler) --
            b0_act(1)  # ACT builds sample 1's B0 while DVE runs the head
            for r in range(HT):
                b0_row(0, r)
                fwd(0, r)
                bwd(0, r)
                g_transpose_row(0, r)
            square_evac(0)
            x_transpose(0)
            sig_evac(0)

            fwd(1, 0)
            bwd(1, 0)
            g_transpose_row(1, 0)
            fwd(1, 1)
            bwd(1, 1)
            g_transpose_row(1, 1)

            bias_build(0, 1, nc.vector)
            for dd in range(2, R + 1):
                bias_build(0, dd, nc.gpsimd)
            for c in range(HT):
                stage_b_col(0, c)
                sqrt_col(0, c)

            fwd(1, 2)
            bwd(1, 2)
            g_transpose_row(1, 2)
            square_evac(1)
            x_transpose(1)
            sig_evac(1)

            for c in range(HT):
                product_col(0, c)
            nc.sync.dma_start(o_sum[:, 0:HT], rowsum[:, 0:HT])

            bias_build(1, 1, nc.vector)
            for dd in range(2, R + 1):
                bias_build(1, dd, nc.gpsimd)
            for c in range(HT):
                stage_b_col(1, c)
                sqrt_col(1, c)
                product_col(1, c)
            nc.scalar.dma_start(o_sum[:, HT:2 * HT], rowsum[:, HT:2 * HT])

    return nc


_KERNEL_CACHE = {}


def _get_kernel(R):
    if R not in _KERNEL_CACHE:
        _KERNEL_CACHE[R] = _build(R)
    return _KERNEL_CACHE[R]


def _exact_row_dist(fg):
    """Exact 1D row distances (distance to nearest fg in the same row),
    float64, BIG-ish large where a row has no fg. fg: [B, H, W] bool."""
    Bn, Hn, Wn = fg.shape
    BIGV = 1.0e9
    col = np.arange(Wn, dtype=np.float64)
    left = np.where(fg, col, -BIGV)
    np.maximum.accumulate(left, axis=2, out=left)
    d_left = col[None, None, :] - left
    right = np.where(fg, -col, -BIGV)[:, :, ::-1]
    np.maximum.accumulate(right, axis=2, out=right)
    d_right = (-right[:, :, ::-1]) - col[None, None, :]
    return np.minimum(d_left, d_right)


def _pick_R(fg):
    """Smallest column-window radius R whose windowed pass equals the exact
    EDT, verified by the sound criterion max(d2_R) < (R+1)^2 (then every
    pixel's optimal |dh| < R+1, so the window covers the true optimum)."""
    g = _exact_row_dist(fg)
    g2 = g * g
    R = 2
    while True:
        d2 = g2.copy()
        for d in range(1, R + 1):
            dd = float(d * d)
            d2[:, :H - d, :] = np.minimum(d2[:, :H - d, :], g2[:, d:, :] + dd)
            d2[:, d:, :] = np.minimum(d2[:, d:, :], g2[:, :H - d, :] + dd)
        if d2.max() < (R + 1) ** 2 or R >= H - 1:
            return R
        R = min(max(R * 2, R + 1), H - 1)


def kernel(logits, targets):
    logits = np.ascontiguousarray(np.asarray(logits, dtype=np.float32))
    targets = np.ascontiguousarray(np.asarray(targets, dtype=np.int32))

    fg = targets[:, 0] > 0
    host_extra = 0.0
    empty = ~fg.any(axis=(1, 2))
    if empty.any():
        # no foreground anywhere: the reference's clipped row-scan gives
        # dist(i,j) = H+W - j. Contribute |sigmoid - 0| * dist on the host
        # and neutralize the sample on device (all-fg -> dist 0).
        dist_empty = REF_BIG - np.arange(W, dtype=np.float64)[None, :]
        for s in np.nonzero(empty)[0]:
            p = 1.0 / (1.0 + np.exp(-logits[s, 0].astype(np.float64)))
            host_extra += float((p * dist_empty).sum())
        targets = targets.copy()
        targets[empty] = 1
        fg = targets[:, 0] > 0

    R = _pick_R(fg)
    import ml_dtypes

    targets_bf16 = np.ascontiguousarray(targets.astype(ml_dtypes.bfloat16))
    logits_bf16 = np.ascontiguousarray(logits.astype(ml_dtypes.bfloat16))
    trace = bool(os.environ.get("BASS_TRACE"))
    nc = _get_kernel(R)
    in_maps = [
        {
            "logits": logits_bf16[i * SPC:(i + 1) * SPC],
            "targets": targets_bf16[i * SPC:(i + 1) * SPC],
        }
        for i in range(N_CORES)
    ]
    res = run_bass_kernel_spmd(nc, in_maps, core_ids=list(range(N_CORES)),
                               trace=trace)
    global LAST_RESULTS
    LAST_RESULTS = res

    total = sum(
        float(np.asarray(r["o_sum"], dtype=np.float64).sum())
        for r in res.results
    ) + host_extra
    return np.float32(total / (B * H * W))


# revision 10
# speedup vs baseline: 1.1258x; 1.0346x over previous
"""Trainium2 Bass kernel for the boundary loss:

    loss = mean_b mean_hw( |sigmoid(logits) - targets| * EDT(targets) )

where EDT is the exact Euclidean distance transform of the background.

Pipeline (per sample, H=W=384, bf16 throughout):
  1. Row pass: exact 1D row distances g via two chamfer scans per row
     (tensor_tensor_scan: state = min(state+1, B0[t]); the backward pass
     runs on the forward result through reversed access patterns). Exact
     for any distance, so no window radius is needed along W.
  2. PE transposes g; the PSUM evacuation applies Square on the ACT
     engine, producing g^2 in transposed layout (h along the free dim)
     with R pad columns preset to BIG.
  3. Column pass: windowed min-plus over |dh| <= R
     (d2 = min_dh dh^2 + g2[h+dh]) using per-offset bias tiles
     (tensor_scalar, 4x DVE mode; d>=2 builds on the otherwise idle
     GpSimd engine) and tensor_tensor mins (2x DVE mode), split per
     128-column block so the chain pipelines. R comes from a sound
     host-side validation: if the exact max d2 < (R+1)^2, every pixel's
     optimum lies inside the window, so the windowed result is the exact
     EDT. Random 0/1 targets give R = 2.
  4. The product stays in transposed space (sum is layout-invariant):
     logits are PE-transposed too, sigmoid rides the PSUM evacuation,
     dist = ACT sqrt of d2, and per-column-block scalar_tensor_tensor
     products emit accum columns summed on the host. No back-transpose.

All values that can win a min are small integers (<= 2R^2 for R <= 11),
exact in bf16; larger R falls back to an f32 build.

Sharding: data-parallel over batch, 2 samples per NeuronCore on 8 cores;
each core emits per-(partition, c-block) sums, the host adds them up.
"""
import os
import sys

sys.path.insert(0, "/opt/trn_rl_repo")

import numpy as np

import concourse.bass as bass
from concourse import masks, mybir
from concourse.bass_utils import run_bass_kernel_spmd
from concourse.tile import TileContext, ScopedClock

F32 = mybir.dt.float32
BF16 = mybir.dt.bfloat16
AF = mybir.ActivationFunctionType
OP = mybir.AluOpType

N_CORES = 8
B, H, W = 16, 384, 384
SPC = B // N_CORES  # samples per core
P = 128
HT = H // P  # 128-row blocks per sample
NF = HT * W  # free elements per fused tile
REF_BIG = float(H + W)  # reference clips distances to this for fg-free samples

LAST_RESULTS = None

# ---------------------------------------------------------------------------
# Walrus in this container rejects >1 sync-wait per instruction ("Too many
# sync wait commands").  Keep the last wait on the instruction and move the
# rest onto same-engine NOPs inserted right before it.
_UID = [0]


def _split_excess_waits(nc, max_waits=1):
    for f in nc.m.functions:
        for bb in f.blocks:
            out = []
            changed = False
            for inst in bb.instructions:
                si = getattr(inst, "sync_info", None)
                waits = list(si.on_wait) if si is not None and si.on_wait else []
                if len(waits) > max_waits:
                    for w in waits[:-max_waits]:
                        _UID[0] += 1
                        nop = mybir.InstNoOp(name=f"I-waitsplit-{_UID[0]}")
                        nop.engine = inst.engine
                        nop.sync_info = mybir.SyncInfo(on_wait=[w], on_update=[])
                        nc.register_instruction(nop)
                        out.append(nop)
                    inst.sync_info = mybir.SyncInfo(
                        on_wait=waits[-max_waits:],
                        on_update=list(si.on_update) if si.on_update else [],
                    )
                    changed = True
                out.append(inst)
            if changed:
                bb.instructions = out


def _split_drain_and_barrier(self, tick_clock, wait_clock):
    nc = self.nc
    drain_inst = nc.sync.drain()
    wait_clock.add_sem_waits(
        drain_inst.ins, ScopedClock({None: tick_clock.global_clock})
    )
    nc.all_engine_barrier()
    assert self.sems is not None
    popped = nc._tile_sem_poison_stack.pop()
    assert popped is self._sem_poison
    nc.clear_and_free_semaphores(list(self.sems.allocated().values()))
    nc.all_engine_barrier()
    _split_excess_waits(nc)


TileContext._drain_and_barrier = _split_drain_and_barrier
# ---------------------------------------------------------------------------


def _build(R):
    """Per-core SPMD kernel with column-window radius R."""
    EDT = BF16 if R <= 11 else F32
    BIG = 16384.0 if R <= 11 else 1.0e7
    WP = W + 2 * R  # padded transposed row length
    nc = bass.Bass("TRN2", target_bir_lowering=False, debug=False,
                   num_devices=N_CORES)
    lg = nc.dram_tensor("logits", [SPC, 1, H, W], BF16, kind="ExternalInput").ap()
    tg = nc.dram_tensor("targets", [SPC, 1, H, W], BF16, kind="ExternalInput").ap()
    o_sum = nc.dram_tensor("o_sum", [P, 1 + HT], F32,
                           kind="ExternalOutput").ap()

    def dram_tile(t, s):
        return t[s, 0].rearrange("(r p) w -> p r w", p=P)

    def r3(tile):  # [P, (r w)] -> [P, r, w]
        return tile[:].rearrange("p (r w) -> p r w", w=W)

    with TileContext(nc) as tc:
        with (
            tc.tile_pool(name="const", bufs=1) as cpool,
            tc.tile_pool(name="work", bufs=1) as wk,
            tc.tile_pool(name="psA", bufs=1, space="PSUM") as psA,
            tc.tile_pool(name="psX", bufs=1, space="PSUM") as psX,
        ):
            ident = cpool.tile([P, P], EDT, tag="ident", name="ident")
            masks.make_identity(nc, ident[:])
            ones = cpool.tile([P, HT * (W + 1)], EDT, tag="ones", name="ones")
            nc.gpsimd.memset(ones[:], 1.0)
            rowsum = cpool.tile([P, 1 + HT], F32, tag="rowsum", name="rowsum")

            T = []
            for s in range(SPC):
                d = {}
                d["tg"] = wk.tile([P, NF], BF16, tag=f"tg{s}", name=f"tg{s}")
                d["x"] = wk.tile([P, NF], BF16, tag=f"x{s}", name=f"x{s}")
                # sample 1's B0 is the padded multiplicative form (1 - tg)
                # with a BIG column between rows, so one flat forward scan
                # covers all three rows (state resets across the pad)
                if s == 0:
                    d["b0"] = wk.tile([P, NF], EDT, tag=f"b0{s}", name=f"b0{s}")
                    d["df"] = wk.tile([P, NF], EDT, tag=f"df{s}", name=f"df{s}")
                else:
                    d["b0"] = wk.tile([P, HT, W + 1], EDT, tag=f"b0{s}",
                                      name=f"b0{s}")
                    d["df"] = wk.tile([P, HT, W + 1], EDT, tag=f"df{s}",
                                      name=f"df{s}")
                    nc.gpsimd.memset(d["b0"][:][:, :, W:W + 1], BIG)
                d["g"] = wk.tile([P, NF], EDT, tag=f"g{s}", name=f"g{s}")
                d["g2t"] = wk.tile([P, HT, WP], EDT, tag=f"g2t{s}",
                                   name=f"g2t{s}")
                d["bias"] = [
                    wk.tile([P, HT, WP], EDT, tag=f"bs{s}_{dd}",
                            name=f"bs{s}_{dd}")
                    for dd in range(1, R + 1)
                ]
                d["m"] = wk.tile([P, NF], EDT, tag=f"m{s}", name=f"m{s}")
                d["d2t"] = wk.tile([P, NF], EDT, tag=f"d2t{s}", name=f"d2t{s}")
                d["dist"] = wk.tile([P, NF], EDT, tag=f"dist{s}",
                                    name=f"dist{s}")
                d["sigt"] = wk.tile([P, NF], BF16, tag=f"sig{s}",
                                    name=f"sig{s}")
                d["junk"] = wk.tile([P, NF], EDT, tag=f"junk{s}",
                                    name=f"junk{s}")
                d["psA"] = psA.tile([P, NF], EDT, tag=f"psA{s}", name=f"psA{s}")
                d["psX"] = psX.tile([P, NF], BF16, tag=f"psX{s}",
                                    name=f"psX{s}")
                nc.gpsimd.memset(d["g2t"][:][:, :, 0:R], BIG)
                nc.gpsimd.memset(d["g2t"][:][:, :, W + R:WP], BIG)
                T.append(d)

            # ---- input DMA: sample 0 row 0 first (pipeline head), rows 1-2
            # on the other queue so they land in parallel
            nc.sync.dma_start(r3(T[0]["tg"])[:, 0:1], dram_tile(tg, 0)[:, 0:1])
            nc.scalar.dma_start(r3(T[0]["tg"])[:, 1:HT],
                                dram_tile(tg, 0)[:, 1:HT])
            nc.sync.dma_start(r3(T[1]["tg"]), dram_tile(tg, 1))
            nc.scalar.dma_start(r3(T[0]["x"]), dram_tile(lg, 0))
            nc.sync.dma_start(r3(T[1]["x"]), dram_tile(lg, 1))

            def b0_row(s, r):
                nc.vector.tensor_scalar(
                    r3(T[s]["b0"])[:, r:r + 1], r3(T[s]["tg"])[:, r:r + 1],
                    -BIG, BIG, OP.mult, OP.add)

            def b0_act_mult(s):
                # (1 - tg) into the padded tile's row interiors (ACT engine)
                nc.scalar.activation(
                    T[s]["b0"][:][:, :, 0:W], r3(T[s]["tg"]), AF.Copy,
                    bias=1.0, scale=-1.0)

            def fwd(s, r):
                sl = slice(r * W, (r + 1) * W)
                nc.vector.tensor_tensor_scan(
                    T[s]["df"][:][:, sl], ones[:, 0:W], T[s]["b0"][:][:, sl],
                    BIG, OP.add, OP.min)

            def fwd_flat(s):
                n = HT * (W + 1)
                b0f = T[s]["b0"][:].rearrange("p r w -> p (r w)")
                dff = T[s]["df"][:].rearrange("p r w -> p (r w)")
                nc.vector.tensor_tensor_scan(
                    dff, ones[:, 0:n], b0f, BIG, OP.add, OP.mult)

            def bwd(s, r):
                if s == 0:
                    dfr = T[s]["df"][:][:, r * W:(r + 1) * W]
                else:
                    dfr = T[s]["df"][:][:, r:r + 1, 0:W].rearrange(
                        "p a w -> p (a w)")
                nc.vector.tensor_tensor_scan(
                    T[s]["g"][:][:, r * W:(r + 1) * W][:, ::-1], ones[:, 0:W],
                    dfr[:, ::-1], BIG, OP.add, OP.min)

            def g_transpose_row(s, r):
                gv = r3(T[s]["g"])
                psv = r3(T[s]["psA"])  # [p, c, h]
                for c in range(HT):
                    nc.tensor.transpose(
                        psv[:, c, r * P:(r + 1) * P],
                        gv[:, r, c * P:(c + 1) * P], ident[:])

            def x_transpose(s):
                xv = r3(T[s]["x"])
                psv = r3(T[s]["psX"])
                for c in range(HT):
                    for r in range(HT):
                        nc.tensor.transpose(
                            psv[:, c, r * P:(r + 1) * P],
                            xv[:, r, c * P:(c + 1) * P], ident[:])

            def square_evac(s):
                nc.scalar.activation(
                    T[s]["g2t"][:][:, :, R:R + W], r3(T[s]["psA"]), AF.Square)

            def sig_evac(s):
                nc.scalar.activation(r3(T[s]["sigt"]), r3(T[s]["psX"]),
                                     AF.Sigmoid)

            def bias_build(s, dd, eng):
                if eng is nc.scalar:
                    nc.scalar.activation(T[s]["bias"][dd - 1][:],
                                         T[s]["g2t"][:], AF.Copy,
                                         bias=float(dd * dd))
                else:
                    eng.tensor_scalar(
                        T[s]["bias"][dd - 1][:], T[s]["g2t"][:],
                        float(dd * dd), None, OP.add)

            def stage_b(s):
                """d2t = min_{|dh|<=R} dh^2 + g2t[h+dh], full width."""
                g2tc = T[s]["g2t"][:][:, :, R:R + W]
                mv = r3(T[s]["m"])
                d2v = r3(T[s]["d2t"])
                for dd in range(1, R + 1):
                    Bv = T[s]["bias"][dd - 1][:]
                    lo, hi = R - dd, R + dd
                    nc.vector.tensor_tensor(
                        mv if dd == 1 else d2v,
                        Bv[:, :, lo:lo + W], Bv[:, :, hi:hi + W], OP.min)
                    if dd == 1:
                        nc.vector.tensor_tensor(mv, mv, g2tc, OP.min)
                    else:
                        nc.vector.tensor_tensor(d2v, mv, d2v, OP.min)
                        if dd < R:
                            nc.vector.tensor_copy(mv, d2v)
                if R == 1:
                    nc.vector.tensor_copy(d2v, mv)

            def sqrt_full(s):
                nc.scalar.activation(T[s]["dist"][:], T[s]["d2t"][:], AF.Sqrt)

            def sqrt_col(s, c):
                nc.scalar.activation(
                    r3(T[s]["dist"])[:, c:c + 1], r3(T[s]["d2t"])[:, c:c + 1],
                    AF.Sqrt)

            def product_acc_act(s, col):
                # tt product on DVE (2x), sum on the ACT accumulator
                nc.vector.tensor_tensor(T[s]["junk"][:], T[s]["sigt"][:],
                                        T[s]["dist"][:], OP.mult)
                nc.scalar.activation(T[s]["m"][:], T[s]["junk"][:], AF.Copy,
                                     accum_out=rowsum[:, col:col + 1])

            def product_col(s, c, col):
                nc.vector.scalar_tensor_tensor(
                    r3(T[s]["junk"])[:, c:c + 1], r3(T[s]["sigt"])[:, c:c + 1],
                    1.0, r3(T[s]["dist"])[:, c:c + 1], OP.mult, OP.mult,
                    accum_out=rowsum[:, col:col + 1])

            # ---------------- schedule (issue order biases the scheduler) --
            b0_act_mult(1)  # ACT builds sample 1's B0 while DVE runs the head
            for r in range(HT):
                b0_row(0, r)
                fwd(0, r)
                bwd(0, r)
                g_transpose_row(0, r)
            square_evac(0)
            x_transpose(0)
            sig_evac(0)

            # all of s1's scans next: sq1 is ready the moment s0's stage B
            # ends, so s1's stage B starts without a stall
            fwd_flat(1)
            for r in range(HT):
                bwd(1, r)
                g_transpose_row(1, r)
            square_evac(1)
            x_transpose(1)
            sig_evac(1)

            for dd in range(2, R + 1):
                bias_build(0, dd, nc.gpsimd)
            bias_build(0, 1, nc.vector)
            stage_b(0)
            sqrt_full(0)

            for dd in range(2, R + 1):
                bias_build(1, dd, nc.gpsimd)
            bias_build(1, 1, nc.vector)
            product_acc_act(0, 0)
            nc.sync.dma_start(o_sum[:, 0:1], rowsum[:, 0:1])
            stage_b(1)
            for c in range(HT):
                sqrt_col(1, c)
                product_col(1, c, 1 + c)
                if c == 1:
                    nc.scalar.dma_start(o_sum[:, 1:HT], rowsum[:, 1:HT])
            nc.sync.dma_start(o_sum[:, HT:1 + HT], rowsum[:, HT:1 + HT])

    return nc


_KERNEL_CACHE = {}


def _get_kernel(R):
    if R not in _KERNEL_CACHE:
        _KERNEL_CACHE[R] = _build(R)
    return _KERNEL_CACHE[R]


def _exact_row_dist(fg):
    """Exact 1D row distances (distance to nearest fg in the same row),
    float64, BIG-ish large where a row has no fg. fg: [B, H, W] bool."""
    Bn, Hn, Wn = fg.shape
    BIGV = 1.0e9
    col = np.arange(Wn, dtype=np.float64)
    left = np.where(fg, col, -BIGV)
    np.maximum.accumulate(left, axis=2, out=left)
    d_left = col[None, None, :] - left
    right = np.where(fg, -col, -BIGV)[:, :, ::-1]
    np.maximum.accumulate(right, axis=2, out=right)
    d_right = (-right[:, :, ::-1]) - col[None, None, :]
    return np.minimum(d_left, d_right)


def _pick_R(fg):
    """Smallest column-window radius R whose windowed pass equals the exact
    EDT, verified by the sound criterion max(d2_R) < (R+1)^2 (then every
    pixel's optimal |dh| < R+1, so the window covers the true optimum)."""
    g = _exact_row_dist(fg)
    g2 = g * g
    R = 2
    while True:
        d2 = g2.copy()
        for d in range(1, R + 1):
            dd = float(d * d)
            d2[:, :H - d, :] = np.minimum(d2[:, :H - d, :], g2[:, d:, :] + dd)
            d2[:, d:, :] = np.minimum(d2[:, d:, :], g2[:, :H - d, :] + dd)
        if d2.max() < (R + 1) ** 2 or R >= H - 1:
            return R
        R = min(max(R * 2, R + 1), H - 1)


def kernel(logits, targets):
    logits = np.ascontiguousarray(np.asarray(logits, dtype=np.float32))
    targets = np.ascontiguousarray(np.asarray(targets, dtype=np.int32))

    fg = targets[:, 0] > 0
    host_extra = 0.0
    empty = ~fg.any(axis=(1, 2))
    if empty.any():
        # no foreground anywhere: the reference's clipped row-scan gives
        # dist(i,j) = H+W - j. Contribute |sigmoid - 0| * dist on the host
        # and neutralize the sample on device (all-fg -> dist 0).
        dist_empty = REF_BIG - np.arange(W, dtype=np.float64)[None, :]
        for s in np.nonzero(empty)[0]:
            p = 1.0 / (1.0 + np.exp(-logits[s, 0].astype(np.float64)))
            host_extra += float((p * dist_empty).sum())
        targets = targets.copy()
        targets[empty] = 1
        fg = targets[:, 0] > 0

    R = _pick_R(fg)
    import ml_dtypes

    targets_bf16 = np.ascontiguousarray(targets.astype(ml_dtypes.bfloat16))
    logits_bf16 = np.ascontiguousarray(logits.astype(ml_dtypes.bfloat16))
    trace = bool(os.environ.get("BASS_TRACE"))
    nc = _get_kernel(R)
    in_maps = [
        {
            "logits": logits_bf16[i * SPC:(i + 1) * SPC],
            "targets": targets_bf16[i * SPC:(i + 1) * SPC],
        }
        for i in range(N_CORES)
    ]
    res = run_bass_kernel_spmd(nc, in_maps, core_ids=list(range(N_CORES)),
                               trace=trace)
    global LAST_RESULTS
    LAST_RESULTS = res

    total = sum(
        float(np.asarray(r["o_sum"], dtype=np.float64).sum())
        for r in res.results
    ) + host_extra
    return np.float32(total / (B * H * W))


# revision 11
# speedup vs baseline: 1.1698x; 1.0391x over previous
"""Trainium2 Bass kernel for the boundary loss:

    loss = mean_b mean_hw( |sigmoid(logits) - targets| * EDT(targets) )

where EDT is the exact Euclidean distance transform of the background.

Pipeline (per sample, H=W=384, bf16 throughout):
  1. Row pass: exact 1D row distances g via two chamfer scans per row
     (tensor_tensor_scan: state = min(state+1, B0[t]); the backward pass
     runs on the forward result through reversed access patterns). Exact
     for any distance, so no window radius is needed along W.
  2. PE transposes g; the PSUM evacuation applies Square on the ACT
     engine, producing g^2 in transposed layout (h along the free dim)
     with R pad columns preset to BIG.
  3. Column pass: windowed min-plus over |dh| <= R
     (d2 = min_dh dh^2 + g2[h+dh]) using per-offset bias tiles
     (tensor_scalar, 4x DVE mode; d>=2 builds on the otherwise idle
     GpSimd engine) and tensor_tensor mins (2x DVE mode), split per
     128-column block so the chain pipelines. R comes from a sound
     host-side validation: if the exact max d2 < (R+1)^2, every pixel's
     optimum lies inside the window, so the windowed result is the exact
     EDT. Random 0/1 targets give R = 2.
  4. The product stays in transposed space (sum is layout-invariant):
     logits are PE-transposed too, sigmoid rides the PSUM evacuation,
     dist = ACT sqrt of d2, and per-column-block scalar_tensor_tensor
     products emit accum columns summed on the host. No back-transpose.

All values that can win a min are small integers (<= 2R^2 for R <= 11),
exact in bf16; larger R falls back to an f32 build.

Sharding: data-parallel over batch, 2 samples per NeuronCore on 8 cores;
each core emits per-(partition, c-block) sums, the host adds them up.
"""
import os
import sys

sys.path.insert(0, "/opt/trn_rl_repo")

import numpy as np

import concourse.bass as bass
from concourse import masks, mybir
from concourse.bass_utils import run_bass_kernel_spmd
from concourse.tile import TileContext, ScopedClock

F32 = mybir.dt.float32
BF16 = mybir.dt.bfloat16
AF = mybir.ActivationFunctionType
OP = mybir.AluOpType

N_CORES = 8
B, H, W = 16, 384, 384
SPC = B // N_CORES  # samples per core
P = 128
HT = H // P  # 128-row blocks per sample
NF = HT * W  # free elements per fused tile
REF_BIG = float(H + W)  # reference clips distances to this for fg-free samples

LAST_RESULTS = None

# ---------------------------------------------------------------------------
# Walrus in this container rejects >1 sync-wait per instruction ("Too many
# sync wait commands").  Keep the last wait on the instruction and move the
# rest onto same-engine NOPs inserted right before it.
_UID = [0]


def _split_excess_waits(nc, max_waits=1):
    for f in nc.m.functions:
        for bb in f.blocks:
            out = []
            changed = False
            for inst in bb.instructions:
                si = getattr(inst, "sync_info", None)
                waits = list(si.on_wait) if si is not None and si.on_wait else []
                if len(waits) > max_waits:
                    for w in waits[:-max_waits]:
                        _UID[0] += 1
                        nop = mybir.InstNoOp(name=f"I-waitsplit-{_UID[0]}")
                        nop.engine = inst.engine
                        nop.sync_info = mybir.SyncInfo(on_wait=[w], on_update=[])
                        nc.register_instruction(nop)
                        out.append(nop)
                    inst.sync_info = mybir.SyncInfo(
                        on_wait=waits[-max_waits:],
                        on_update=list(si.on_update) if si.on_update else [],
                    )
                    changed = True
                out.append(inst)
            if changed:
                bb.instructions = out


def _split_drain_and_barrier(self, tick_clock, wait_clock):
    nc = self.nc
    drain_inst = nc.sync.drain()
    wait_clock.add_sem_waits(
        drain_inst.ins, ScopedClock({None: tick_clock.global_clock})
    )
    nc.all_engine_barrier()
    assert self.sems is not None
    popped = nc._tile_sem_poison_stack.pop()
    assert popped is self._sem_poison
    nc.clear_and_free_semaphores(list(self.sems.allocated().values()))
    nc.all_engine_barrier()
    _split_excess_waits(nc)


TileContext._drain_and_barrier = _split_drain_and_barrier
# ---------------------------------------------------------------------------


def _build(R):
    """Per-core SPMD kernel with column-window radius R."""
    EDT = BF16 if R <= 11 else F32
    BIG = 16384.0 if R <= 11 else 1.0e7
    WP = W + 2 * R  # padded transposed row length
    nc = bass.Bass("TRN2", target_bir_lowering=False, debug=False,
                   num_devices=N_CORES)
    lg = nc.dram_tensor("logits", [SPC, 1, H, W], BF16, kind="ExternalInput").ap()
    tg = nc.dram_tensor("targets", [SPC, 1, H, W], BF16, kind="ExternalInput").ap()
    o_sum = nc.dram_tensor("o_sum", [P, 1 + HT], F32,
                           kind="ExternalOutput").ap()

    def dram_tile(t, s):
        return t[s, 0].rearrange("(r p) w -> p r w", p=P)

    def r3(tile):  # [P, (r w)] -> [P, r, w]
        return tile[:].rearrange("p (r w) -> p r w", w=W)

    with TileContext(nc) as tc:
        with (
            tc.tile_pool(name="const", bufs=1) as cpool,
            tc.tile_pool(name="work", bufs=1) as wk,
            tc.tile_pool(name="psA", bufs=1, space="PSUM") as psA,
            tc.tile_pool(name="psX", bufs=1, space="PSUM") as psX,
        ):
            ident = cpool.tile([P, P], EDT, tag="ident", name="ident")
            masks.make_identity(nc, ident[:])
            ones = cpool.tile([P, HT * (W + 1)], EDT, tag="ones", name="ones")
            nc.gpsimd.memset(ones[:], 1.0)
            rowsum = cpool.tile([P, 1 + HT], F32, tag="rowsum", name="rowsum")

            T = []
            for s in range(SPC):
                d = {}
                d["tg"] = wk.tile([P, NF], BF16, tag=f"tg{s}", name=f"tg{s}")
                d["x"] = wk.tile([P, NF], BF16, tag=f"x{s}", name=f"x{s}")
                # sample 1's B0 is the padded multiplicative form (1 - tg)
                # with a BIG column between rows, so one flat forward scan
                # covers all three rows (state resets across the pad)
                if s == 0:
                    d["b0"] = wk.tile([P, NF], EDT, tag=f"b0{s}", name=f"b0{s}")
                    d["df"] = wk.tile([P, NF], EDT, tag=f"df{s}", name=f"df{s}")
                else:
                    d["b0"] = wk.tile([P, HT, W + 1], EDT, tag=f"b0{s}",
                                      name=f"b0{s}")
                    d["df"] = wk.tile([P, HT, W + 1], EDT, tag=f"df{s}",
                                      name=f"df{s}")
                    nc.gpsimd.memset(d["b0"][:][:, :, W:W + 1], BIG)
                d["g"] = wk.tile([P, NF], EDT, tag=f"g{s}", name=f"g{s}")
                d["g2t"] = wk.tile([P, HT, WP], EDT, tag=f"g2t{s}",
                                   name=f"g2t{s}")
                d["bias"] = [
                    wk.tile([P, HT, WP], EDT, tag=f"bs{s}_{dd}",
                            name=f"bs{s}_{dd}")
                    for dd in range(1, R + 1)
                ]
                d["m"] = wk.tile([P, NF], EDT, tag=f"m{s}", name=f"m{s}")
                d["d2t"] = wk.tile([P, NF], EDT, tag=f"d2t{s}", name=f"d2t{s}")
                d["dist"] = wk.tile([P, NF], EDT, tag=f"dist{s}",
                                    name=f"dist{s}")
                d["sigt"] = wk.tile([P, NF], BF16, tag=f"sig{s}",
                                    name=f"sig{s}")
                d["junk"] = wk.tile([P, NF], EDT, tag=f"junk{s}",
                                    name=f"junk{s}")
                d["psA"] = psA.tile([P, NF], EDT, tag=f"psA{s}", name=f"psA{s}")
                d["psX"] = psX.tile([P, NF], BF16, tag=f"psX{s}",
                                    name=f"psX{s}")
                nc.gpsimd.memset(d["g2t"][:][:, :, 0:R], BIG)
                nc.gpsimd.memset(d["g2t"][:][:, :, W + R:WP], BIG)
                T.append(d)

            # ---- input DMA: sample 0 row 0 first (pipeline head), rows 1-2
            # on the other queue so they land in parallel
            nc.sync.dma_start(r3(T[0]["tg"])[:, 0:1], dram_tile(tg, 0)[:, 0:1])
            nc.scalar.dma_start(r3(T[0]["tg"])[:, 1:HT],
                                dram_tile(tg, 0)[:, 1:HT])
            nc.sync.dma_start(r3(T[1]["tg"]), dram_tile(tg, 1))
            nc.scalar.dma_start(r3(T[0]["x"]), dram_tile(lg, 0))
            nc.sync.dma_start(r3(T[1]["x"]), dram_tile(lg, 1))

            def b0_row(s, r):
                nc.vector.tensor_scalar(
                    r3(T[s]["b0"])[:, r:r + 1], r3(T[s]["tg"])[:, r:r + 1],
                    -BIG, BIG, OP.mult, OP.add)

            def b0_act_mult(s):
                # (1 - tg) into the padded tile's row interiors (ACT engine)
                nc.scalar.activation(
                    T[s]["b0"][:][:, :, 0:W], r3(T[s]["tg"]), AF.Copy,
                    bias=1.0, scale=-1.0)

            def fwd(s, r):
                sl = slice(r * W, (r + 1) * W)
                nc.vector.tensor_tensor_scan(
                    T[s]["df"][:][:, sl], ones[:, 0:W], T[s]["b0"][:][:, sl],
                    BIG, OP.add, OP.min)

            def fwd_flat(s):
                n = HT * (W + 1)
                b0f = T[s]["b0"][:].rearrange("p r w -> p (r w)")
                dff = T[s]["df"][:].rearrange("p r w -> p (r w)")
                nc.vector.tensor_tensor_scan(
                    dff, ones[:, 0:n], b0f, BIG, OP.add, OP.mult)

            def bwd(s, r):
                if s == 0:
                    dfr = T[s]["df"][:][:, r * W:(r + 1) * W]
                else:
                    dfr = T[s]["df"][:][:, r:r + 1, 0:W].rearrange(
                        "p a w -> p (a w)")
                nc.vector.tensor_tensor_scan(
                    T[s]["g"][:][:, r * W:(r + 1) * W][:, ::-1], ones[:, 0:W],
                    dfr[:, ::-1], BIG, OP.add, OP.min)

            def g_transpose_row(s, r):
                gv = r3(T[s]["g"])
                psv = r3(T[s]["psA"])  # [p, c, h]
                for c in range(HT):
                    nc.tensor.transpose(
                        psv[:, c, r * P:(r + 1) * P],
                        gv[:, r, c * P:(c + 1) * P], ident[:])

            def x_transpose(s):
                xv = r3(T[s]["x"])
                psv = r3(T[s]["psX"])
                for c in range(HT):
                    for r in range(HT):
                        nc.tensor.transpose(
                            psv[:, c, r * P:(r + 1) * P],
                            xv[:, r, c * P:(c + 1) * P], ident[:])

            def square_evac(s):
                nc.scalar.activation(
                    T[s]["g2t"][:][:, :, R:R + W], r3(T[s]["psA"]), AF.Square)

            def sig_evac(s):
                nc.scalar.activation(r3(T[s]["sigt"]), r3(T[s]["psX"]),
                                     AF.Sigmoid)

            def bias_build(s, dd, eng):
                if eng is nc.scalar:
                    nc.scalar.activation(T[s]["bias"][dd - 1][:],
                                         T[s]["g2t"][:], AF.Copy,
                                         bias=float(dd * dd))
                else:
                    eng.tensor_scalar(
                        T[s]["bias"][dd - 1][:], T[s]["g2t"][:],
                        float(dd * dd), None, OP.add)

            def stage_b(s):
                """d2t = min_{|dh|<=R} dh^2 + g2t[h+dh], full width."""
                g2tc = T[s]["g2t"][:][:, :, R:R + W]
                mv = r3(T[s]["m"])
                d2v = r3(T[s]["d2t"])
                for dd in range(1, R + 1):
                    Bv = T[s]["bias"][dd - 1][:]
                    lo, hi = R - dd, R + dd
                    nc.vector.tensor_tensor(
                        mv if dd == 1 else d2v,
                        Bv[:, :, lo:lo + W], Bv[:, :, hi:hi + W], OP.min)
                    if dd == 1:
                        nc.vector.tensor_tensor(mv, mv, g2tc, OP.min)
                    else:
                        nc.vector.tensor_tensor(d2v, mv, d2v, OP.min)
                        if dd < R:
                            nc.vector.tensor_copy(mv, d2v)
                if R == 1:
                    nc.vector.tensor_copy(d2v, mv)

            def sqrt_full(s):
                nc.scalar.activation(T[s]["dist"][:], T[s]["d2t"][:], AF.Sqrt)

            def sqrt_col(s, c):
                nc.scalar.activation(
                    r3(T[s]["dist"])[:, c:c + 1], r3(T[s]["d2t"])[:, c:c + 1],
                    AF.Sqrt)

            def product_acc_act(s, col):
                # tt product on DVE (2x), sum on the ACT accumulator
                nc.vector.tensor_tensor(T[s]["junk"][:], T[s]["sigt"][:],
                                        T[s]["dist"][:], OP.mult)
                nc.scalar.activation(T[s]["m"][:], T[s]["junk"][:], AF.Copy,
                                     accum_out=rowsum[:, col:col + 1])

            def product_col(s, c, col):
                nc.vector.scalar_tensor_tensor(
                    r3(T[s]["junk"])[:, c:c + 1], r3(T[s]["sigt"])[:, c:c + 1],
                    1.0, r3(T[s]["dist"])[:, c:c + 1], OP.mult, OP.mult,
                    accum_out=rowsum[:, col:col + 1])

            def product_full(s, col):
                nc.vector.scalar_tensor_tensor(
                    T[s]["junk"][:], T[s]["sigt"][:], 1.0, T[s]["dist"][:],
                    OP.mult, OP.mult, accum_out=rowsum[:, col:col + 1])

            # ---------------- schedule (issue order biases the scheduler) --
            # ACT program: b0(1), sq0, sq1, sigT0, sqrt0, sigT1, sqrt1 cols.
            # Nothing ACT-side in the tail: all products are DVE stt ops.
            b0_act_mult(1)  # ACT builds sample 1's B0 while DVE runs the head
            for r in range(HT):
                b0_row(0, r)
                fwd(0, r)
                bwd(0, r)
                g_transpose_row(0, r)
            square_evac(0)
            x_transpose(0)

            # all of s1's scans next: sq1 is ready the moment s0's stage B
            # ends, so s1's stage B starts without a stall
            fwd_flat(1)
            for r in range(HT):
                bwd(1, r)
                g_transpose_row(1, r)
            square_evac(1)
            sig_evac(0)
            x_transpose(1)

            for dd in range(2, R + 1):
                bias_build(0, dd, nc.gpsimd)
            bias_build(0, 1, nc.vector)
            stage_b(0)
            sqrt_full(0)
            sig_evac(1)

            for dd in range(2, R + 1):
                bias_build(1, dd, nc.gpsimd)
            bias_build(1, 1, nc.vector)
            stage_b(1)
            product_full(0, 0)
            nc.sync.dma_start(o_sum[:, 0:1], rowsum[:, 0:1])
            for c in range(HT):
                sqrt_col(1, c)
                product_col(1, c, 1 + c)
                if c == 1:
                    nc.scalar.dma_start(o_sum[:, 1:HT], rowsum[:, 1:HT])
            nc.sync.dma_start(o_sum[:, HT:1 + HT], rowsum[:, HT:1 + HT])

    return nc


_KERNEL_CACHE = {}


def _get_kernel(R):
    if R not in _KERNEL_CACHE:
        _KERNEL_CACHE[R] = _build(R)
    return _KERNEL_CACHE[R]


def _exact_row_dist(fg):
    """Exact 1D row distances (distance to nearest fg in the same row),
    float64, BIG-ish large where a row has no fg. fg: [B, H, W] bool."""
    Bn, Hn, Wn = fg.shape
    BIGV = 1.0e9
    col = np.arange(Wn, dtype=np.float64)
    left = np.where(fg, col, -BIGV)
    np.maximum.accumulate(left, axis=2, out=left)
    d_left = col[None, None, :] - left
    right = np.where(fg, -col, -BIGV)[:, :, ::-1]
    np.maximum.accumulate(right, axis=2, out=right)
    d_right = (-right[:, :, ::-1]) - col[None, None, :]
    return np.minimum(d_left, d_right)


def _pick_R(fg):
    """Smallest column-window radius R whose windowed pass equals the exact
    EDT, verified by the sound criterion max(d2_R) < (R+1)^2 (then every
    pixel's optimal |dh| < R+1, so the window covers the true optimum)."""
    g = _exact_row_dist(fg)
    g2 = g * g
    R = 2
    while True:
        d2 = g2.copy()
        for d in range(1, R + 1):
            dd = float(d * d)
            d2[:, :H - d, :] = np.minimum(d2[:, :H - d, :], g2[:, d:, :] + dd)
            d2[:, d:, :] = np.minimum(d2[:, d:, :], g2[:, :H - d, :] + dd)
        if d2.max() < (R + 1) ** 2 or R >= H - 1:
            return R
        R = min(max(R * 2, R + 1), H - 1)


def kernel(logits, targets):
    logits = np.ascontiguousarray(np.asarray(logits, dtype=np.float32))
    targets = np.ascontiguousarray(np.asarray(targets, dtype=np.int32))

    fg = targets[:, 0] > 0
    host_extra = 0.0
    empty = ~fg.any(axis=(1, 2))
    if empty.any():
        # no foreground anywhere: the reference's clipped row-scan gives
        # dist(i,j) = H+W - j. Contribute |sigmoid - 0| * dist on the host
        # and neutralize the sample on device (all-fg -> dist 0).
        dist_empty = REF_BIG - np.arange(W, dtype=np.float64)[None, :]
        for s in np.nonzero(empty)[0]:
            p = 1.0 / (1.0 + np.exp(-logits[s, 0].astype(np.float64)))
            host_extra += float((p * dist_empty).sum())
        targets = targets.copy()
        targets[empty] = 1
        fg = targets[:, 0] > 0

    R = _pick_R(fg)
    import ml_dtypes

    targets_bf16 = np.ascontiguousarray(targets.astype(ml_dtypes.bfloat16))
    logits_bf16 = np.ascontiguousarray(logits.astype(ml_dtypes.bfloat16))
    trace = bool(os.environ.get("BASS_TRACE"))
    nc = _get_kernel(R)
    in_maps = [
        {
            "logits": logits_bf16[i * SPC:(i + 1) * SPC],
            "targets": targets_bf16[i * SPC:(i + 1) * SPC],
        }
        for i in range(N_CORES)
    ]
    res = run_bass_kernel_spmd(nc, in_maps, core_ids=list(range(N_CORES)),
                               trace=trace)
    global LAST_RESULTS
    LAST_RESULTS = res

    total = sum(
        float(np.asarray(r["o_sum"], dtype=np.float64).sum())
        for r in res.results
    ) + host_extra
    return np.float32(total / (B * H * W))


# revision 12
# speedup vs baseline: 1.1709x; 1.0009x over previous
"""Trainium2 Bass kernel for the boundary loss:

    loss = mean_b mean_hw( |sigmoid(logits) - targets| * EDT(targets) )

where EDT is the exact Euclidean distance transform of the background.

Pipeline (per sample, H=W=384, bf16 throughout):
  1. Row pass: exact 1D row distances g via two chamfer scans per row
     (tensor_tensor_scan: state = min(state+1, B0[t]); the backward pass
     runs on the forward result through reversed access patterns). Exact
     for any distance, so no window radius is needed along W.
  2. PE transposes g; the PSUM evacuation applies Square on the ACT
     engine, producing g^2 in transposed layout (h along the free dim)
     with R pad columns preset to BIG.
  3. Column pass: windowed min-plus over |dh| <= R
     (d2 = min_dh dh^2 + g2[h+dh]) using per-offset bias tiles
     (tensor_scalar, 4x DVE mode; d>=2 builds on the otherwise idle
     GpSimd engine) and tensor_tensor mins (2x DVE mode), split per
     128-column block so the chain pipelines. R comes from a sound
     host-side validation: if the exact max d2 < (R+1)^2, every pixel's
     optimum lies inside the window, so the windowed result is the exact
     EDT. Random 0/1 targets give R = 2.
  4. The product stays in transposed space (sum is layout-invariant):
     logits are PE-transposed too, sigmoid rides the PSUM evacuation,
     dist = ACT sqrt of d2, and per-column-block scalar_tensor_tensor
     products emit accum columns summed on the host. No back-transpose.

All values that can win a min are small integers (<= 2R^2 for R <= 11),
exact in bf16; larger R falls back to an f32 build.

Sharding: data-parallel over batch, 2 samples per NeuronCore on 8 cores;
each core emits per-(partition, c-block) sums, the host adds them up.
"""
import os
import sys

sys.path.insert(0, "/opt/trn_rl_repo")

import numpy as np

import concourse.bass as bass
from concourse import masks, mybir
from concourse.bass_utils import run_bass_kernel_spmd
from concourse.tile import TileContext, ScopedClock

F32 = mybir.dt.float32
BF16 = mybir.dt.bfloat16
AF = mybir.ActivationFunctionType
OP = mybir.AluOpType

N_CORES = 8
B, H, W = 16, 384, 384
SPC = B // N_CORES  # samples per core
P = 128
HT = H // P  # 128-row blocks per sample
NF = HT * W  # free elements per fused tile
REF_BIG = float(H + W)  # reference clips distances to this for fg-free samples

LAST_RESULTS = None

# ---------------------------------------------------------------------------
# Walrus in this container rejects >1 sync-wait per instruction ("Too many
# sync wait commands").  Keep the last wait on the instruction and move the
# rest onto same-engine NOPs inserted right before it.
_UID = [0]


def _split_excess_waits(nc, max_waits=1):
    for f in nc.m.functions:
        for bb in f.blocks:
            out = []
            changed = False
            for inst in bb.instructions:
                si = getattr(inst, "sync_info", None)
                waits = list(si.on_wait) if si is not None and si.on_wait else []
                if len(waits) > max_waits:
                    for w in waits[:-max_waits]:
                        _UID[0] += 1
                        nop = mybir.InstNoOp(name=f"I-waitsplit-{_UID[0]}")
                        nop.engine = inst.engine
                        nop.sync_info = mybir.SyncInfo(on_wait=[w], on_update=[])
                        nc.register_instruction(nop)
                        out.append(nop)
                    inst.sync_info = mybir.SyncInfo(
                        on_wait=waits[-max_waits:],
                        on_update=list(si.on_update) if si.on_update else [],
                    )
                    changed = True
                out.append(inst)
            if changed:
                bb.instructions = out


def _split_drain_and_barrier(self, tick_clock, wait_clock):
    nc = self.nc
    drain_inst = nc.sync.drain()
    wait_clock.add_sem_waits(
        drain_inst.ins, ScopedClock({None: tick_clock.global_clock})
    )
    nc.all_engine_barrier()
    assert self.sems is not None
    popped = nc._tile_sem_poison_stack.pop()
    assert popped is self._sem_poison
    nc.clear_and_free_semaphores(list(self.sems.allocated().values()))
    nc.all_engine_barrier()
    _split_excess_waits(nc)


TileContext._drain_and_barrier = _split_drain_and_barrier
# ---------------------------------------------------------------------------


def _build(R):
    """Per-core SPMD kernel with column-window radius R."""
    EDT = BF16 if R <= 11 else F32
    BIG = 16384.0 if R <= 11 else 1.0e7
    WP = W + 2 * R  # padded transposed row length
    nc = bass.Bass("TRN2", target_bir_lowering=False, debug=False,
                   num_devices=N_CORES)
    lg = nc.dram_tensor("logits", [SPC, 1, H, W], BF16, kind="ExternalInput").ap()
    tg = nc.dram_tensor("targets", [SPC, 1, H, W], BF16, kind="ExternalInput").ap()
    o_sum = nc.dram_tensor("o_sum", [P, 1 + HT], F32,
                           kind="ExternalOutput").ap()

    def dram_tile(t, s):
        return t[s, 0].rearrange("(r p) w -> p r w", p=P)

    def r3(tile):  # [P, (r w)] -> [P, r, w]
        return tile[:].rearrange("p (r w) -> p r w", w=W)

    with TileContext(nc) as tc:
        with (
            tc.tile_pool(name="const", bufs=1) as cpool,
            tc.tile_pool(name="work", bufs=1) as wk,
            tc.tile_pool(name="psA", bufs=1, space="PSUM") as psA,
            tc.tile_pool(name="psX", bufs=1, space="PSUM") as psX,
        ):
            ident = cpool.tile([P, P], EDT, tag="ident", name="ident")
            masks.make_identity(nc, ident[:])
            ones = cpool.tile([P, HT * (W + 1)], EDT, tag="ones", name="ones")
            nc.gpsimd.memset(ones[:], 1.0)
            rowsum = cpool.tile([P, 1 + HT], F32, tag="rowsum", name="rowsum")

            T = []
            for s in range(SPC):
                d = {}
                d["tg"] = wk.tile([P, NF], BF16, tag=f"tg{s}", name=f"tg{s}")
                d["x"] = wk.tile([P, NF], BF16, tag=f"x{s}", name=f"x{s}")
                # sample 1's B0 is the padded multiplicative form (1 - tg)
                # with a BIG column between rows, so one flat forward scan
                # covers all three rows (state resets across the pad)
                if s == 0:
                    d["b0"] = wk.tile([P, NF], EDT, tag=f"b0{s}", name=f"b0{s}")
                    d["df"] = wk.tile([P, NF], EDT, tag=f"df{s}", name=f"df{s}")
                else:
                    d["b0"] = wk.tile([P, HT, W + 1], EDT, tag=f"b0{s}",
                                      name=f"b0{s}")
                    d["df"] = wk.tile([P, HT, W + 1], EDT, tag=f"df{s}",
                                      name=f"df{s}")
                    nc.gpsimd.memset(d["b0"][:][:, :, W:W + 1], BIG)
                d["g"] = wk.tile([P, NF], EDT, tag=f"g{s}", name=f"g{s}")
                d["g2t"] = wk.tile([P, HT, WP], EDT, tag=f"g2t{s}",
                                   name=f"g2t{s}")
                d["bias"] = [
                    wk.tile([P, HT, WP], EDT, tag=f"bs{s}_{dd}",
                            name=f"bs{s}_{dd}")
                    for dd in range(1, R + 1)
                ]
                d["m"] = wk.tile([P, NF], EDT, tag=f"m{s}", name=f"m{s}")
                d["d2t"] = wk.tile([P, NF], EDT, tag=f"d2t{s}", name=f"d2t{s}")
                d["dist"] = wk.tile([P, NF], EDT, tag=f"dist{s}",
                                    name=f"dist{s}")
                d["sigt"] = wk.tile([P, NF], BF16, tag=f"sig{s}",
                                    name=f"sig{s}")
                d["junk"] = wk.tile([P, NF], EDT, tag=f"junk{s}",
                                    name=f"junk{s}")
                d["psA"] = psA.tile([P, NF], EDT, tag=f"psA{s}", name=f"psA{s}")
                d["psX"] = psX.tile([P, NF], BF16, tag=f"psX{s}",
                                    name=f"psX{s}")
                nc.gpsimd.memset(d["g2t"][:][:, :, 0:R], BIG)
                nc.gpsimd.memset(d["g2t"][:][:, :, W + R:WP], BIG)
                T.append(d)

            # ---- input DMA: sample 0 row 0 first (pipeline head), rows 1-2
            # on the other queue so they land in parallel
            nc.sync.dma_start(r3(T[0]["tg"])[:, 0:1], dram_tile(tg, 0)[:, 0:1])
            nc.scalar.dma_start(r3(T[0]["tg"])[:, 1:HT],
                                dram_tile(tg, 0)[:, 1:HT])
            nc.sync.dma_start(r3(T[1]["tg"]), dram_tile(tg, 1))
            nc.scalar.dma_start(r3(T[0]["x"]), dram_tile(lg, 0))
            nc.sync.dma_start(r3(T[1]["x"]), dram_tile(lg, 1))

            def b0_row(s, r):
                nc.vector.tensor_scalar(
                    r3(T[s]["b0"])[:, r:r + 1], r3(T[s]["tg"])[:, r:r + 1],
                    -BIG, BIG, OP.mult, OP.add)

            def b0_act_mult(s):
                # (1 - tg) into the padded tile's row interiors (ACT engine)
                nc.scalar.activation(
                    T[s]["b0"][:][:, :, 0:W], r3(T[s]["tg"]), AF.Copy,
                    bias=1.0, scale=-1.0)

            def fwd(s, r):
                sl = slice(r * W, (r + 1) * W)
                nc.vector.tensor_tensor_scan(
                    T[s]["df"][:][:, sl], ones[:, 0:W], T[s]["b0"][:][:, sl],
                    BIG, OP.add, OP.min)

            def fwd_flat(s):
                n = HT * (W + 1)
                b0f = T[s]["b0"][:].rearrange("p r w -> p (r w)")
                dff = T[s]["df"][:].rearrange("p r w -> p (r w)")
                nc.vector.tensor_tensor_scan(
                    dff, ones[:, 0:n], b0f, BIG, OP.add, OP.mult)

            def bwd(s, r):
                if s == 0:
                    dfr = T[s]["df"][:][:, r * W:(r + 1) * W]
                else:
                    dfr = T[s]["df"][:][:, r:r + 1, 0:W].rearrange(
                        "p a w -> p (a w)")
                nc.vector.tensor_tensor_scan(
                    T[s]["g"][:][:, r * W:(r + 1) * W][:, ::-1], ones[:, 0:W],
                    dfr[:, ::-1], BIG, OP.add, OP.min)

            def g_transpose_row(s, r):
                gv = r3(T[s]["g"])
                psv = r3(T[s]["psA"])  # [p, c, h]
                for c in range(HT):
                    nc.tensor.transpose(
                        psv[:, c, r * P:(r + 1) * P],
                        gv[:, r, c * P:(c + 1) * P], ident[:])

            def x_transpose(s):
                xv = r3(T[s]["x"])
                psv = r3(T[s]["psX"])
                for c in range(HT):
                    for r in range(HT):
                        nc.tensor.transpose(
                            psv[:, c, r * P:(r + 1) * P],
                            xv[:, r, c * P:(c + 1) * P], ident[:])

            def square_evac(s):
                nc.scalar.activation(
                    T[s]["g2t"][:][:, :, R:R + W], r3(T[s]["psA"]), AF.Square)

            def sig_evac(s):
                nc.scalar.activation(r3(T[s]["sigt"]), r3(T[s]["psX"]),
                                     AF.Sigmoid)

            def bias_build(s, dd, eng):
                if eng is nc.scalar:
                    nc.scalar.activation(T[s]["bias"][dd - 1][:],
                                         T[s]["g2t"][:], AF.Copy,
                                         bias=float(dd * dd))
                else:
                    eng.tensor_scalar(
                        T[s]["bias"][dd - 1][:], T[s]["g2t"][:],
                        float(dd * dd), None, OP.add)

            def stage_b(s):
                """d2t = min_{|dh|<=R} dh^2 + g2t[h+dh], full width."""
                g2tc = T[s]["g2t"][:][:, :, R:R + W]
                mv = r3(T[s]["m"])
                d2v = r3(T[s]["d2t"])
                for dd in range(1, R + 1):
                    Bv = T[s]["bias"][dd - 1][:]
                    lo, hi = R - dd, R + dd
                    nc.vector.tensor_tensor(
                        mv if dd == 1 else d2v,
                        Bv[:, :, lo:lo + W], Bv[:, :, hi:hi + W], OP.min)
                    if dd == 1:
                        nc.vector.tensor_tensor(mv, mv, g2tc, OP.min)
                    else:
                        nc.vector.tensor_tensor(d2v, mv, d2v, OP.min)
                        if dd < R:
                            nc.vector.tensor_copy(mv, d2v)
                if R == 1:
                    nc.vector.tensor_copy(d2v, mv)

            def sqrt_full(s):
                nc.scalar.activation(T[s]["dist"][:], T[s]["d2t"][:], AF.Sqrt)

            def sqrt_col(s, c):
                nc.scalar.activation(
                    r3(T[s]["dist"])[:, c:c + 1], r3(T[s]["d2t"])[:, c:c + 1],
                    AF.Sqrt)

            def product_acc_act(s, col):
                # tt product on DVE (2x), sum on the ACT accumulator
                nc.vector.tensor_tensor(T[s]["junk"][:], T[s]["sigt"][:],
                                        T[s]["dist"][:], OP.mult)
                nc.scalar.activation(T[s]["m"][:], T[s]["junk"][:], AF.Copy,
                                     accum_out=rowsum[:, col:col + 1])

            def product_col(s, c, col):
                nc.vector.scalar_tensor_tensor(
                    r3(T[s]["junk"])[:, c:c + 1], r3(T[s]["sigt"])[:, c:c + 1],
                    1.0, r3(T[s]["dist"])[:, c:c + 1], OP.mult, OP.mult,
                    accum_out=rowsum[:, col:col + 1])

            def product_full(s, col):
                nc.vector.scalar_tensor_tensor(
                    T[s]["junk"][:], T[s]["sigt"][:], 1.0, T[s]["dist"][:],
                    OP.mult, OP.mult, accum_out=rowsum[:, col:col + 1])

            # ---------------- schedule (issue order biases the scheduler) --
            # ACT program: b0(1), sq0, sq1, sigT0, sqrt0, sigT1, sqrt1 cols.
            # Nothing ACT-side in the tail: all products are DVE stt ops.
            b0_act_mult(1)  # ACT builds sample 1's B0 while DVE runs the head
            for r in range(HT):
                b0_row(0, r)
                fwd(0, r)
                bwd(0, r)
                g_transpose_row(0, r)
            square_evac(0)
            x_transpose(0)

            # all of s1's scans next: sq1 is ready the moment s0's stage B
            # ends, so s1's stage B starts without a stall
            fwd_flat(1)
            for r in range(HT):
                bwd(1, r)
                g_transpose_row(1, r)
            square_evac(1)
            x_transpose(1)

            for dd in range(1, R + 1):
                bias_build(0, dd, nc.vector)
            stage_b(0)
            sqrt_full(0)
            sig_evac(0)

            for dd in range(1, R + 1):
                bias_build(1, dd, nc.vector)
            stage_b(1)
            sig_evac(1)
            product_full(0, 0)
            nc.sync.dma_start(o_sum[:, 0:1], rowsum[:, 0:1])
            for c in range(HT):
                sqrt_col(1, c)
                product_col(1, c, 1 + c)
                if c == 1:
                    nc.scalar.dma_start(o_sum[:, 1:HT], rowsum[:, 1:HT])
            nc.sync.dma_start(o_sum[:, HT:1 + HT], rowsum[:, HT:1 + HT])

    return nc


_KERNEL_CACHE = {}


def _get_kernel(R):
    if R not in _KERNEL_CACHE:
        _KERNEL_CACHE[R] = _build(R)
    return _KERNEL_CACHE[R]


def _exact_row_dist(fg):
    """Exact 1D row distances (distance to nearest fg in the same row),
    float64, BIG-ish large where a row has no fg. fg: [B, H, W] bool."""
    Bn, Hn, Wn = fg.shape
    BIGV = 1.0e9
    col = np.arange(Wn, dtype=np.float64)
    left = np.where(fg, col, -BIGV)
    np.maximum.accumulate(left, axis=2, out=left)
    d_left = col[None, None, :] - left
    right = np.where(fg, -col, -BIGV)[:, :, ::-1]
    np.maximum.accumulate(right, axis=2, out=right)
    d_right = (-right[:, :, ::-1]) - col[None, None, :]
    return np.minimum(d_left, d_right)


def _pick_R(fg):
    """Smallest column-window radius R whose windowed pass equals the exact
    EDT, verified by the sound criterion max(d2_R) < (R+1)^2 (then every
    pixel's optimal |dh| < R+1, so the window covers the true optimum)."""
    g = _exact_row_dist(fg)
    g2 = g * g
    R = 2
    while True:
        d2 = g2.copy()
        for d in range(1, R + 1):
            dd = float(d * d)
            d2[:, :H - d, :] = np.minimum(d2[:, :H - d, :], g2[:, d:, :] + dd)
            d2[:, d:, :] = np.minimum(d2[:, d:, :], g2[:, :H - d, :] + dd)
        if d2.max() < (R + 1) ** 2 or R >= H - 1:
            return R
        R = min(max(R * 2, R + 1), H - 1)


def kernel(logits, targets):
    logits = np.ascontiguousarray(np.asarray(logits, dtype=np.float32))
    targets = np.ascontiguousarray(np.asarray(targets, dtype=np.int32))

    fg = targets[:, 0] > 0
    host_extra = 0.0
    empty = ~fg.any(axis=(1, 2))
    if empty.any():
        # no foreground anywhere: the reference's clipped row-scan gives
        # dist(i,j) = H+W - j. Contribute |sigmoid - 0| * dist on the host
        # and neutralize the sample on device (all-fg -> dist 0).
        dist_empty = REF_BIG - np.arange(W, dtype=np.float64)[None, :]
        for s in np.nonzero(empty)[0]:
            p = 1.0 / (1.0 + np.exp(-logits[s, 0].astype(np.float64)))
            host_extra += float((p * dist_empty).sum())
        targets = targets.copy()
        targets[empty] = 1
        fg = targets[:, 0] > 0

    R = _pick_R(fg)
    import ml_dtypes

    targets_bf16 = np.ascontiguousarray(targets.astype(ml_dtypes.bfloat16))
    logits_bf16 = np.ascontiguousarray(logits.astype(ml_dtypes.bfloat16))
    trace = bool(os.environ.get("BASS_TRACE"))
    nc = _get_kernel(R)
    in_maps = [
        {
            "logits": logits_bf16[i * SPC:(i + 1) * SPC],
            "targets": targets_bf16[i * SPC:(i + 1) * SPC],
        }
        for i in range(N_CORES)
    ]
    res = run_bass_kernel_spmd(nc, in_maps, core_ids=list(range(N_CORES)),
                               trace=trace)
    global LAST_RESULTS
    LAST_RESULTS = res

    total = sum(
        float(np.asarray(r["o_sum"], dtype=np.float64).sum())
        for r in res.results
    ) + host_extra
    return np.float32(total / (B * H * W))


# revision 13
# speedup vs baseline: 1.2125x; 1.0355x over previous
"""Trainium2 Bass kernel for the boundary loss:

    loss = mean_b mean_hw( |sigmoid(logits) - targets| * EDT(targets) )

where EDT is the exact Euclidean distance transform of the background.

Pipeline (per sample, H=W=384, bf16 throughout):
  1. Row pass: exact 1D row distances g via two chamfer scans per row
     (tensor_tensor_scan: state = min(state+1, B0[t]); the backward pass
     runs on the forward result through reversed access patterns). Exact
     for any distance, so no window radius is needed along W.
  2. PE transposes g; the PSUM evacuation applies Square on the ACT
     engine, producing g^2 in transposed layout (h along the free dim)
     with R pad columns preset to BIG.
  3. Column pass: windowed min-plus over |dh| <= R
     (d2 = min_dh dh^2 + g2[h+dh]) using per-offset bias tiles
     (tensor_scalar, 4x DVE mode; d>=2 builds on the otherwise idle
     GpSimd engine) and tensor_tensor mins (2x DVE mode), split per
     128-column block so the chain pipelines. R comes from a sound
     host-side validation: if the exact max d2 < (R+1)^2, every pixel's
     optimum lies inside the window, so the windowed result is the exact
     EDT. Random 0/1 targets give R = 2.
  4. The product stays in transposed space (sum is layout-invariant):
     logits are PE-transposed too, sigmoid rides the PSUM evacuation,
     dist = ACT sqrt of d2, and per-column-block scalar_tensor_tensor
     products emit accum columns summed on the host. No back-transpose.

All values that can win a min are small integers (<= 2R^2 for R <= 11),
exact in bf16; larger R falls back to an f32 build.

Sharding: data-parallel over batch, 2 samples per NeuronCore on 8 cores;
each core emits per-(partition, c-block) sums, the host adds them up.
"""
import os
import sys

sys.path.insert(0, "/opt/trn_rl_repo")

import numpy as np

import concourse.bass as bass
from concourse import masks, mybir
from concourse.bass_utils import run_bass_kernel_spmd
from concourse.tile import TileContext, ScopedClock

F32 = mybir.dt.float32
BF16 = mybir.dt.bfloat16
AF = mybir.ActivationFunctionType
OP = mybir.AluOpType

N_CORES = 8
B, H, W = 16, 384, 384
SPC = B // N_CORES  # samples per core
P = 128
HT = H // P  # 128-row blocks per sample
NF = HT * W  # free elements per fused tile
REF_BIG = float(H + W)  # reference clips distances to this for fg-free samples

LAST_RESULTS = None

# ---------------------------------------------------------------------------
# Walrus in this container rejects >1 sync-wait per instruction ("Too many
# sync wait commands").  Keep the last wait on the instruction and move the
# rest onto same-engine NOPs inserted right before it.
_UID = [0]


def _split_excess_waits(nc, max_waits=1):
    for f in nc.m.functions:
        for bb in f.blocks:
            out = []
            changed = False
            for inst in bb.instructions:
                si = getattr(inst, "sync_info", None)
                waits = list(si.on_wait) if si is not None and si.on_wait else []
                if len(waits) > max_waits:
                    for w in waits[:-max_waits]:
                        _UID[0] += 1
                        nop = mybir.InstNoOp(name=f"I-waitsplit-{_UID[0]}")
                        nop.engine = inst.engine
                        nop.sync_info = mybir.SyncInfo(on_wait=[w], on_update=[])
                        nc.register_instruction(nop)
                        out.append(nop)
                    inst.sync_info = mybir.SyncInfo(
                        on_wait=waits[-max_waits:],
                        on_update=list(si.on_update) if si.on_update else [],
                    )
                    changed = True
                out.append(inst)
            if changed:
                bb.instructions = out


def _split_drain_and_barrier(self, tick_clock, wait_clock):
    nc = self.nc
    drain_inst = nc.sync.drain()
    wait_clock.add_sem_waits(
        drain_inst.ins, ScopedClock({None: tick_clock.global_clock})
    )
    nc.all_engine_barrier()
    assert self.sems is not None
    popped = nc._tile_sem_poison_stack.pop()
    assert popped is self._sem_poison
    nc.clear_and_free_semaphores(list(self.sems.allocated().values()))
    nc.all_engine_barrier()
    _split_excess_waits(nc)


TileContext._drain_and_barrier = _split_drain_and_barrier
# ---------------------------------------------------------------------------


def _build(R):
    """Per-core SPMD kernel with column-window radius R."""
    EDT = BF16 if R <= 11 else F32
    BIG = 16384.0 if R <= 11 else 1.0e7
    WP = W + 2 * R  # padded transposed row length
    nc = bass.Bass("TRN2", target_bir_lowering=False, debug=False,
                   num_devices=N_CORES)
    lg = nc.dram_tensor("logits", [SPC, 1, H, W], BF16, kind="ExternalInput").ap()
    tg = nc.dram_tensor("targets", [SPC, 1, H, W], BF16, kind="ExternalInput").ap()
    o_sum = nc.dram_tensor("o_sum", [P, 1 + HT], F32,
                           kind="ExternalOutput").ap()

    def dram_tile(t, s):
        return t[s, 0].rearrange("(r p) w -> p r w", p=P)

    def r3(tile):  # [P, (r w)] -> [P, r, w]
        return tile[:].rearrange("p (r w) -> p r w", w=W)

    with TileContext(nc) as tc:
        with (
            tc.tile_pool(name="const", bufs=1) as cpool,
            tc.tile_pool(name="work", bufs=1) as wk,
            tc.tile_pool(name="psA", bufs=1, space="PSUM") as psA,
        ):
            ident = cpool.tile([P, P], EDT, tag="ident", name="ident")
            masks.make_identity(nc, ident[:])
            ones = cpool.tile([P, HT * (W + 1)], EDT, tag="ones", name="ones")
            nc.gpsimd.memset(ones[:], 1.0)
            rowsum = cpool.tile([P, 1 + HT], F32, tag="rowsum", name="rowsum")

            T = []
            for s in range(SPC):
                d = {}
                d["tg"] = wk.tile([P, NF], BF16, tag=f"tg{s}", name=f"tg{s}")
                d["x"] = wk.tile([P, NF], BF16, tag=f"x{s}", name=f"x{s}")
                # sample 1's B0 is the padded multiplicative form (1 - tg)
                # with a BIG column between rows, so one flat forward scan
                # covers all three rows (state resets across the pad)
                if s == 0:
                    d["b0"] = wk.tile([P, NF], EDT, tag=f"b0{s}", name=f"b0{s}")
                    d["df"] = wk.tile([P, NF], EDT, tag=f"df{s}", name=f"df{s}")
                else:
                    d["b0"] = wk.tile([P, HT, W + 1], EDT, tag=f"b0{s}",
                                      name=f"b0{s}")
                    d["df"] = wk.tile([P, HT, W + 1], EDT, tag=f"df{s}",
                                      name=f"df{s}")
                    nc.gpsimd.memset(d["b0"][:][:, :, W:W + 1], BIG)
                d["g"] = wk.tile([P, NF], EDT, tag=f"g{s}", name=f"g{s}")
                d["g2t"] = wk.tile([P, HT, WP], EDT, tag=f"g2t{s}",
                                   name=f"g2t{s}")
                d["bias"] = [
                    wk.tile([P, HT, WP], EDT, tag=f"bs{s}_{dd}",
                            name=f"bs{s}_{dd}")
                    for dd in range(1, R + 1)
                ]
                d["m"] = wk.tile([P, NF], EDT, tag=f"m{s}", name=f"m{s}")
                d["d2t"] = wk.tile([P, NF], EDT, tag=f"d2t{s}", name=f"d2t{s}")
                d["dist"] = wk.tile([P, NF], EDT, tag=f"dist{s}",
                                    name=f"dist{s}")
                d["sigt"] = wk.tile([P, NF], BF16, tag=f"sig{s}",
                                    name=f"sig{s}")
                d["junk"] = wk.tile([P, NF], EDT, tag=f"junk{s}",
                                    name=f"junk{s}")
                d["psA"] = psA.tile([P, NF], EDT, tag=f"psA{s}", name=f"psA{s}")
                # x transposes reuse the same PSUM tile: the WAR dependency
                # on the Square evacuation keeps the PE from hoisting them
                # ahead of the g transposes (which would delay everything)
                d["psX"] = d["psA"]
                nc.gpsimd.memset(d["g2t"][:][:, :, 0:R], BIG)
                nc.gpsimd.memset(d["g2t"][:][:, :, W + R:WP], BIG)
                T.append(d)

            # ---- input DMA: sample 0 row 0 first (pipeline head), rows 1-2
            # on the other queue so they land in parallel
            nc.sync.dma_start(r3(T[0]["tg"])[:, 0:1], dram_tile(tg, 0)[:, 0:1])
            nc.scalar.dma_start(r3(T[0]["tg"])[:, 1:HT],
                                dram_tile(tg, 0)[:, 1:HT])
            nc.sync.dma_start(r3(T[1]["tg"]), dram_tile(tg, 1))
            nc.scalar.dma_start(r3(T[0]["x"]), dram_tile(lg, 0))
            nc.sync.dma_start(r3(T[1]["x"]), dram_tile(lg, 1))

            def b0_row(s, r):
                nc.vector.tensor_scalar(
                    r3(T[s]["b0"])[:, r:r + 1], r3(T[s]["tg"])[:, r:r + 1],
                    -BIG, BIG, OP.mult, OP.add)

            def b0_act_mult(s):
                # (1 - tg) into the padded tile's row interiors (ACT engine)
                nc.scalar.activation(
                    T[s]["b0"][:][:, :, 0:W], r3(T[s]["tg"]), AF.Copy,
                    bias=1.0, scale=-1.0)

            def fwd(s, r):
                sl = slice(r * W, (r + 1) * W)
                nc.vector.tensor_tensor_scan(
                    T[s]["df"][:][:, sl], ones[:, 0:W], T[s]["b0"][:][:, sl],
                    BIG, OP.add, OP.min)

            def fwd_flat(s):
                n = HT * (W + 1)
                b0f = T[s]["b0"][:].rearrange("p r w -> p (r w)")
                dff = T[s]["df"][:].rearrange("p r w -> p (r w)")
                nc.vector.tensor_tensor_scan(
                    dff, ones[:, 0:n], b0f, BIG, OP.add, OP.mult)

            def bwd(s, r):
                if s == 0:
                    dfr = T[s]["df"][:][:, r * W:(r + 1) * W]
                else:
                    dfr = T[s]["df"][:][:, r:r + 1, 0:W].rearrange(
                        "p a w -> p (a w)")
                nc.vector.tensor_tensor_scan(
                    T[s]["g"][:][:, r * W:(r + 1) * W][:, ::-1], ones[:, 0:W],
                    dfr[:, ::-1], BIG, OP.add, OP.min)

            def g_transpose_row(s, r):
                gv = r3(T[s]["g"])
                psv = r3(T[s]["psA"])  # [p, c, h]
                for c in range(HT):
                    nc.tensor.transpose(
                        psv[:, c, r * P:(r + 1) * P],
                        gv[:, r, c * P:(c + 1) * P], ident[:])

            def x_transpose(s):
                xv = r3(T[s]["x"])
                psv = r3(T[s]["psX"])
                for c in range(HT):
                    for r in range(HT):
                        nc.tensor.transpose(
                            psv[:, c, r * P:(r + 1) * P],
                            xv[:, r, c * P:(c + 1) * P], ident[:])

            def square_evac(s):
                nc.scalar.activation(
                    T[s]["g2t"][:][:, :, R:R + W], r3(T[s]["psA"]), AF.Square)

            def sig_evac(s):
                nc.scalar.activation(r3(T[s]["sigt"]), r3(T[s]["psX"]),
                                     AF.Sigmoid)

            def bias_build(s, dd, eng):
                if eng is nc.scalar:
                    nc.scalar.activation(T[s]["bias"][dd - 1][:],
                                         T[s]["g2t"][:], AF.Copy,
                                         bias=float(dd * dd))
                else:
                    eng.tensor_scalar(
                        T[s]["bias"][dd - 1][:], T[s]["g2t"][:],
                        float(dd * dd), None, OP.add)

            def stage_b(s):
                """d2t = min_{|dh|<=R} dh^2 + g2t[h+dh], full width."""
                g2tc = T[s]["g2t"][:][:, :, R:R + W]
                mv = r3(T[s]["m"])
                d2v = r3(T[s]["d2t"])
                for dd in range(1, R + 1):
                    Bv = T[s]["bias"][dd - 1][:]
                    lo, hi = R - dd, R + dd
                    nc.vector.tensor_tensor(
                        mv if dd == 1 else d2v,
                        Bv[:, :, lo:lo + W], Bv[:, :, hi:hi + W], OP.min)
                    if dd == 1:
                        nc.vector.tensor_tensor(mv, mv, g2tc, OP.min)
                    else:
                        nc.vector.tensor_tensor(d2v, mv, d2v, OP.min)
                        if dd < R:
                            nc.vector.tensor_copy(mv, d2v)
                if R == 1:
                    nc.vector.tensor_copy(d2v, mv)

            def sqrt_full(s):
                nc.scalar.activation(T[s]["dist"][:], T[s]["d2t"][:], AF.Sqrt)

            def sqrt_col(s, c):
                nc.scalar.activation(
                    r3(T[s]["dist"])[:, c:c + 1], r3(T[s]["d2t"])[:, c:c + 1],
                    AF.Sqrt)

            def product_acc_act(s, col):
                # tt product on DVE (2x), sum on the ACT accumulator
                nc.vector.tensor_tensor(T[s]["junk"][:], T[s]["sigt"][:],
                                        T[s]["dist"][:], OP.mult)
                nc.scalar.activation(T[s]["m"][:], T[s]["junk"][:], AF.Copy,
                                     accum_out=rowsum[:, col:col + 1])

            def product_col(s, c, col):
                nc.vector.scalar_tensor_tensor(
                    r3(T[s]["junk"])[:, c:c + 1], r3(T[s]["sigt"])[:, c:c + 1],
                    1.0, r3(T[s]["dist"])[:, c:c + 1], OP.mult, OP.mult,
                    accum_out=rowsum[:, col:col + 1])

            def product_full(s, col):
                nc.vector.scalar_tensor_tensor(
                    T[s]["junk"][:], T[s]["sigt"][:], 1.0, T[s]["dist"][:],
                    OP.mult, OP.mult, accum_out=rowsum[:, col:col + 1])

            # ---------------- schedule (issue order biases the scheduler) --
            # ACT program: b0(1), sq0, sq1, sigT0, sqrt0, sigT1, sqrt1 cols.
            # Nothing ACT-side in the tail: all products are DVE stt ops.
            b0_act_mult(1)  # ACT builds sample 1's B0 while DVE runs the head
            for r in range(HT):
                b0_row(0, r)
                fwd(0, r)
                bwd(0, r)
                g_transpose_row(0, r)
            square_evac(0)
            x_transpose(0)

            # all of s1's scans next: sq1 is ready the moment s0's stage B
            # ends, so s1's stage B starts without a stall
            fwd_flat(1)
            for r in range(HT):
                bwd(1, r)
                g_transpose_row(1, r)
            square_evac(1)
            x_transpose(1)

            for dd in range(1, R + 1):
                bias_build(0, dd, nc.vector)
            stage_b(0)
            sqrt_full(0)
            sig_evac(0)

            for dd in range(1, R + 1):
                bias_build(1, dd, nc.vector)
            stage_b(1)
            sig_evac(1)
            product_full(0, 0)
            nc.sync.dma_start(o_sum[:, 0:1], rowsum[:, 0:1])
            for c in range(HT):
                sqrt_col(1, c)
                product_col(1, c, 1 + c)
                if c == 1:
                    nc.scalar.dma_start(o_sum[:, 1:HT], rowsum[:, 1:HT])
            nc.sync.dma_start(o_sum[:, HT:1 + HT], rowsum[:, HT:1 + HT])

    return nc


_KERNEL_CACHE = {}


def _get_kernel(R):
    if R not in _KERNEL_CACHE:
        _KERNEL_CACHE[R] = _build(R)
    return _KERNEL_CACHE[R]


def _exact_row_dist(fg):
    """Exact 1D row distances (distance to nearest fg in the same row),
    float64, BIG-ish large where a row has no fg. fg: [B, H, W] bool."""
    Bn, Hn, Wn = fg.shape
    BIGV = 1.0e9
    col = np.arange(Wn, dtype=np.float64)
    left = np.where(fg, col, -BIGV)
    np.maximum.accumulate(left, axis=2, out=left)
    d_left = col[None, None, :] - left
    right = np.where(fg, -col, -BIGV)[:, :, ::-1]
    np.maximum.accumulate(right, axis=2, out=right)
    d_right = (-right[:, :, ::-1]) - col[None, None, :]
    return np.minimum(d_left, d_right)


def _pick_R(fg):
    """Smallest column-window radius R whose windowed pass equals the exact
    EDT, verified by the sound criterion max(d2_R) < (R+1)^2 (then every
    pixel's optimal |dh| < R+1, so the window covers the true optimum)."""
    g = _exact_row_dist(fg)
    g2 = g * g
    R = 2
    while True:
        d2 = g2.copy()
        for d in range(1, R + 1):
            dd = float(d * d)
            d2[:, :H - d, :] = np.minimum(d2[:, :H - d, :], g2[:, d:, :] + dd)
            d2[:, d:, :] = np.minimum(d2[:, d:, :], g2[:, :H - d, :] + dd)
        if d2.max() < (R + 1) ** 2 or R >= H - 1:
            return R
        R = min(max(R * 2, R + 1), H - 1)


def kernel(logits, targets):
    logits = np.ascontiguousarray(np.asarray(logits, dtype=np.float32))
    targets = np.ascontiguousarray(np.asarray(targets, dtype=np.int32))

    fg = targets[:, 0] > 0
    host_extra = 0.0
    empty = ~fg.any(axis=(1, 2))
    if empty.any():
        # no foreground anywhere: the reference's clipped row-scan gives
        # dist(i,j) = H+W - j. Contribute |sigmoid - 0| * dist on the host
        # and neutralize the sample on device (all-fg -> dist 0).
        dist_empty = REF_BIG - np.arange(W, dtype=np.float64)[None, :]
        for s in np.nonzero(empty)[0]:
            p = 1.0 / (1.0 + np.exp(-logits[s, 0].astype(np.float64)))
            host_extra += float((p * dist_empty).sum())
        targets = targets.copy()
        targets[empty] = 1
        fg = targets[:, 0] > 0

    R = _pick_R(fg)
    import ml_dtypes

    targets_bf16 = np.ascontiguousarray(targets.astype(ml_dtypes.bfloat16))
    logits_bf16 = np.ascontiguousarray(logits.astype(ml_dtypes.bfloat16))
    trace = bool(os.environ.get("BASS_TRACE"))
    nc = _get_kernel(R)
    in_maps = [
        {
            "logits": logits_bf16[i * SPC:(i + 1) * SPC],
            "targets": targets_bf16[i * SPC:(i + 1) * SPC],
        }
        for i in range(N_CORES)
    ]
    res = run_bass_kernel_spmd(nc, in_maps, core_ids=list(range(N_CORES)),
                               trace=trace)
    global LAST_RESULTS
    LAST_RESULTS = res

    total = sum(
        float(np.asarray(r["o_sum"], dtype=np.float64).sum())
        for r in res.results
    ) + host_extra
    return np.float32(total / (B * H * W))
